# revision 15
# baseline (speedup 1.0000x reference)
"""MoE-routed multi-head attention kernel for 8 Trainium2 NeuronCores.

Problem shape (hardcoded):
  query/key/value: [4, 2048, 512] f32
  Wg [512,8], Wk/Wv [512,64], Wq [8,512,64], Wo [8,64,512], biases.
  TOP_K=2 routed experts act as the two attention heads.

Sharding: core c = 2*b + h handles batch b, query-half h (1024 query tokens),
with the full 2048 keys/values of batch b.

Key structural choices (v2):
  - bk is dropped: adding bk to kh shifts every logit of a query by a
    constant, which softmax ignores.  bv is folded into bo on the host
    (bo' = bo + bv @ Wo_e) since attn rows sum to 1 before the gate.
  - vh is computed directly key-major (vT chunks stationary, Wv moving),
    so no PE transposes of vh are needed.
  - Router logits (3-term bf16 hi/lo, fp32-exact for top-2 selection) are
    computed T-layout with tiny Wg-stationary loads, transposed per qt.
  - The expert gather (qsel from q_all) is a chain of 8 fused
    scalar_tensor_tensor ops per head; the expert scatter (attn -> cm)
    is a gpsimd local_scatter with per-token int16 indices.
  - Phase C: per kc one [128,1024] fp32 2-bank PSUM logits tile (both
    heads via row-grouped concurrent matmuls), one [128,1024] exp, and
    2 attention matmuls, software-pipelined.  B-blocks for qts 4-7 are
    interleaved into C(half0)'s PE stream, D-blocks into C(half1)'s, so
    no engine idles and the PE never sees a matmul-free window (HAM).
  - Output is written bf16; host casts to fp32.
"""

import numpy as np

import concourse.bass as bass
import concourse.mybir as mybir
import concourse.tile as tile
from concourse import bacc
from concourse import bass_utils
from concourse.masks import make_identity

P = 128
D = 512          # d_model
T = 2048         # kv tokens per core (full batch)
NQ = 1024        # query tokens per core
E = 8            # experts
DK = 64          # head dim
DC = D // P      # 4 contraction chunks
NKC = T // P     # 16 key chunks
NQT = NQ // P    # 8 query tiles
VW = DK + 1      # vh columns + ones column (denominator trick)
HD = 512         # phase-C column granularity (half of NQ)

FP = mybir.dt.float32
U32 = mybir.dt.uint32
I16 = mybir.dt.int16
BF = mybir.dt.bfloat16
AF = mybir.ActivationFunctionType
OP = mybir.AluOpType
AX = mybir.AxisListType

USE_GPSIMD_SCATTER = True
ACT_ACCUM = True         # activation accum_out for softmax row sums

# ---- w0a packed layout (bf16 columns) ----
_W0A = {}
_off = 0
for name, cols in [("Wk2s", DC * P), ("Wv", DC * DK), ("Wg_hi", DC * E),
                   ("Wg_lo", DC * E), ("ones", P), ("bq_row", D), ("bo", D)]:
    _W0A[name] = _off
    _off += cols
_W0A_COLS = _off
_W0B_COLS = 2 * DC * D   # Wq_f (e d) | Wo_f (e d)


def _emit(nc, tc, ctx):
    const = ctx.enter_context(tc.tile_pool(name="const", bufs=1))
    persist = ctx.enter_context(tc.tile_pool(name="persist", bufs=1))
    work = ctx.enter_context(tc.tile_pool(name="work", bufs=3))
    expp = ctx.enter_context(tc.tile_pool(name="expp", bufs=3))
    dpool = ctx.enter_context(tc.tile_pool(name="dpool", bufs=2))
    ps_log = ctx.enter_context(tc.tile_pool(name="ps_log", bufs=2, space="PSUM"))
    ps_att = ctx.enter_context(tc.tile_pool(name="ps_att", bufs=2, space="PSUM"))
    ps_m = ctx.enter_context(tc.tile_pool(name="ps_m", bufs=2, space="PSUM"))

    dram = {}
    for name, shape, dt in [
        ("w0a", [P, _W0A_COLS], BF), ("w0b", [P, _W0B_COLS], BF),
        ("kTn", [P, DC * T], BF), ("vTn", [P, DC * T], BF),
        ("qhi", [P, DC * NQ], BF), ("qlo", [P, DC * NQ], BF),
    ]:
        dram[name] = nc.dram_tensor(name, shape, dt, kind="ExternalInput").ap()
    out_d = nc.dram_tensor("out", [NQ, D], BF, kind="ExternalOutput").ap()

    # ---- DMAs, ordered by first use; kT split so khT2 starts early ----
    w0a = const.tile([P, _W0A_COLS], BF, tag="w0a")
    nc.sync.dma_start(w0a[:], dram["w0a"])
    kTt = persist.tile([P, DC * T], BF, tag="kTt")
    for j in range(4):
        cs = slice(j * T, (j + 1) * T)
        nc.sync.dma_start(kTt[:, cs], dram["kTn"][:, cs])
    qhi_t = persist.tile([P, DC * NQ], BF, tag="qhi")
    nc.sync.dma_start(qhi_t[:], dram["qhi"])
    qlo_t = persist.tile([P, DC * NQ], BF, tag="qlo")
    nc.sync.dma_start(qlo_t[:], dram["qlo"])
    w0b = const.tile([P, _W0B_COLS], BF, tag="w0b")
    nc.sync.dma_start(w0b[:], dram["w0b"])
    vTt = persist.tile([P, DC * T], BF, tag="vTt")
    for j in range(4):
        cs = slice(j * T, (j + 1) * T)
        nc.sync.dma_start(vTt[:, cs], dram["vTn"][:, cs])

    def w0(name, r0, r1, c0, c1):
        base = _W0A[name]
        return w0a[r0:r1, base + c0:base + c1]

    Wk2s = {dc: w0("Wk2s", 0, P, dc * P, (dc + 1) * P) for dc in range(DC)}
    Wv = {dc: w0("Wv", 0, P, dc * DK, (dc + 1) * DK) for dc in range(DC)}
    Wg_hi = {dc: w0("Wg_hi", 0, P, dc * E, (dc + 1) * E) for dc in range(DC)}
    Wg_lo = {dc: w0("Wg_lo", 0, P, dc * E, (dc + 1) * E) for dc in range(DC)}
    ones_blk = w0("ones", 0, P, 0, P)
    ones_row = w0("ones", 0, 1, 0, P)        # [1,128] lhsT for bias matmul
    bq_row = w0("bq_row", 0, 1, 0, D)        # [1,512] (e d) order
    bo8 = w0("bo", 0, E, 0, D)               # [8,512] bo' = bo + bv@Wo
    Wq_f = {dc: w0b[:, dc * D:(dc + 1) * D] for dc in range(DC)}
    Wo_f = {dc: w0b[:, DC * D + dc * D:DC * D + (dc + 1) * D] for dc in range(DC)}
    qT_hi = {dc: qhi_t[:, dc * NQ:(dc + 1) * NQ] for dc in range(DC)}
    qT_lo = {dc: qlo_t[:, dc * NQ:(dc + 1) * NQ] for dc in range(DC)}

    # ---- constants ----
    ident = const.tile([P, P], FP, tag="ident")
    make_identity(nc, ident[:])
    ident_b = const.tile([P, P], BF, tag="ident_b")
    make_identity(nc, ident_b[:])
    iota8 = const.tile([P, E], FP, tag="iota8")
    nc.gpsimd.iota(iota8[:], pattern=[[1, E]], channel_multiplier=0,
                   allow_small_or_imprecise_dtypes=True)
    iota64x8 = const.tile([P, DK], FP, tag="iota64x8")   # 0,8,16,...,504
    nc.gpsimd.iota(iota64x8[:], pattern=[[8, DK]], channel_multiplier=0,
                   allow_small_or_imprecise_dtypes=True)
    iota_e = const.tile([P, DK * E], BF, tag="iota_e")   # expert id, (d e) cols
    nc.gpsimd.iota(iota_e[:].rearrange("p (d e) -> p d e", e=E),
                   pattern=[[0, DK], [1, E]], channel_multiplier=0,
                   allow_small_or_imprecise_dtypes=True)

    # ---- persistent intermediates ----
    khT2 = persist.tile([P, T], BF, tag="khT2")
    vh_aug = persist.tile([P, NKC * VW], BF, tag="vh_aug")
    routerT = persist.tile([E, NQ], FP, tag="routerT")
    qselT2 = persist.tile([P, NQ], BF, tag="qselT2")
    combT = persist.tile([E, NQ], BF, tag="combT")
    attnT = [persist.tile([VW, NQ], BF, tag=f"attnT{h}", name=f"attnT{h}")
             for h in range(2)]
    g_all = persist.tile([P, 2 * NQT], FP, tag="g_all")    # gates, 2 per qt
    idx16 = persist.tile([P, P * NQT], I16, tag="idx16")   # scatter idxs per qt

    # vh_aug ones columns (col 64 of each kc block)
    nc.vector.tensor_copy(
        vh_aug[:].rearrange("p (c w) -> p c w", w=VW)[:, :, DK],
        ones_blk[:, 0:NKC])

    # ---- A1: khT2 [128, T] (head-doubled kh, pre-scaled by 1/8) ----
    def emit_khT2_chunk(j):
        cs = slice(j * HD, (j + 1) * HD)
        ps = ps_m.tile([P, HD], FP, tag="ps")
        for dc in range(DC):
            nc.tensor.matmul(ps[:], Wk2s[dc][:], kTt[:, j * T + dc * HD:
                                                     j * T + (dc + 1) * HD],
                             start=(dc == 0), stop=(dc == DC - 1))
        nc.vector.tensor_copy(khT2[:, cs], ps[:])

    for j in range(4):
        emit_khT2_chunk(j)

    # ---- A2: router logits, T-layout, 3-term hi/lo (fp32-exact) ----
    for half in range(2):
        hs = slice(half * HD, (half + 1) * HD)
        ps_r = ps_m.tile([E, HD], FP, tag="ps")
        first = True
        for dc in range(DC):
            for wg, q in [(Wg_hi, qT_hi), (Wg_lo, qT_hi), (Wg_hi, qT_lo)]:
                nc.tensor.matmul(ps_r[:], wg[dc][:], q[dc][:, hs],
                                 start=first,
                                 stop=(dc == DC - 1 and q is qT_lo))
                first = False
        nc.vector.tensor_copy(routerT[:, hs], ps_r[:])

    # ---- A3 (interleaved into B blocks): vh key-major -> vh_aug ----
    def emit_vh_kc(kc):
        # shares the ps_a tag: vh use ends before phase C allocates ps_a
        ps = ps_att.tile([P, DK], FP, tag="ps_a", name="ps_vh")
        for dc in range(DC):
            nc.tensor.matmul(ps[:], vTt[:, kc * D + dc * P:kc * D + (dc + 1) * P],
                             Wv[dc][:], start=(dc == 0), stop=(dc == DC - 1))
        nc.vector.tensor_copy(vh_aug[:, kc * VW:kc * VW + DK], ps[:])

    # ---- B block for one qt: q_all, top-2 select, gather, gates ----
    # Split in two: the PE/ACT/DVE producer part, and a deferred finish
    # (the qsel2/comb8 transposes) emitted one interleave slot later so
    # the PE never waits on the DVE gather.
    b_state = {}

    def emit_B_mm(qt):
        qs = slice(qt * P, (qt + 1) * P)
        # PE: all-expert query projection + bq bias row
        ps_qa = ps_m.tile([P, D], FP, tag="ps")
        for dc in range(DC):
            nc.tensor.matmul(ps_qa[:], qT_hi[dc][:, qs], Wq_f[dc][:],
                             start=(dc == 0), stop=False)
        nc.tensor.matmul(ps_qa[:], ones_row, bq_row, start=False, stop=True)
        qa_b = work.tile([P, D], BF, tag="qa_b")
        nc.scalar.activation(qa_b[:], ps_qa[:], AF.Copy)
        # PE: router logits transpose for this qt
        ps_lg = ps_m.tile([P, E], FP, tag="ps")
        nc.tensor.matmul(ps_lg[:], routerT[:, qs], ident[:E, :E],
                         is_transpose=True)
        lg8 = work.tile([P, E], FP, tag="lg8")
        nc.vector.tensor_copy(lg8[:], ps_lg[:])
        # top-2 selection
        m8 = work.tile([P, E], FP, tag="m8")
        nc.vector.max(out=m8[:], in_=lg8[:])
        i8 = work.tile([P, E], U32, tag="i8")
        nc.vector.max_index(i8[:], m8[:], lg8[:])
        if8 = work.tile([P, 2], FP, tag="if8")
        nc.vector.tensor_copy(if8[:], i8[:, 0:2])
        # softmax pieces: e8 + row sum, top-2 exps
        e8 = work.tile([P, E], FP, tag="e8")
        ssum = work.tile([P, 1], FP, tag="ssum")
        if ACT_ACCUM:
            nc.scalar.activation(e8[:], lg8[:], AF.Exp, accum_out=ssum[:])
        else:
            nc.scalar.activation(e8[:], lg8[:], AF.Exp)
            nc.vector.reduce_sum(ssum[:], e8[:], axis=AX.X)
        gtop = work.tile([P, 2], FP, tag="gtop")
        nc.scalar.activation(gtop[:], m8[:, 0:2], AF.Exp)
        srec = work.tile([P, 1], FP, tag="srec")
        nc.vector.reciprocal(srec[:], ssum[:])
        g = g_all[:, 2 * qt:2 * qt + 2]
        nc.vector.tensor_scalar(g, gtop[:], srec[:], None, op0=OP.mult)
        # masked gather: one fused (iota==e_h)*qa per head, then e-reduce
        qsel2 = work.tile([P, P], BF, tag="qsel2")
        for h in range(2):
            u = work.tile([P, D], BF, tag=f"u{h}", name=f"u{h}")
            nc.vector.scalar_tensor_tensor(
                u[:], iota_e[:], if8[:, h:h + 1], qa_b[:],
                op0=OP.is_equal, op1=OP.mult)
            with nc.allow_low_precision(reason="one-hot masked sum: only one "
                                        "of the 8 summands is nonzero"):
                nc.vector.reduce_sum(qsel2[:, h * DK:(h + 1) * DK],
                                     u[:].rearrange("p (d e) -> p d e", e=E),
                                     axis=AX.X)
        # scatter indices for phase D (int16, (d e) order: 8*d + e_h)
        if USE_GPSIMD_SCATTER:
            for h in range(2):
                nc.vector.tensor_scalar(
                    idx16[:, qt * P + h * DK:qt * P + (h + 1) * DK],
                    iota64x8[:], if8[:, h:h + 1], None, op0=OP.add)
        # gate-weighted one-hot combine row (for bo' bias matmul)
        comb8 = work.tile([P, E], BF, tag="comb8")
        tmp8 = work.tile([P, E], BF, tag="tmp8")
        nc.vector.scalar_tensor_tensor(
            comb8[:], iota8[:], if8[:, 0:1],
            g[:, 0:1].to_broadcast((P, E)), op0=OP.is_equal, op1=OP.mult)
        nc.vector.scalar_tensor_tensor(
            tmp8[:], iota8[:], if8[:, 1:2],
            g[:, 1:2].to_broadcast((P, E)), op0=OP.is_equal, op1=OP.mult)
        nc.vector.tensor_tensor(comb8[:], comb8[:], tmp8[:], op=OP.add)
        b_state[qt] = (qsel2, comb8)

    def emit_B_fin(qt):
        qs = slice(qt * P, (qt + 1) * P)
        qsel2, comb8 = b_state.pop(qt)
        ps_qsT = ps_m.tile([P, P], BF, tag="ps")
        nc.tensor.matmul(ps_qsT[:], qsel2[:], ident_b[:], is_transpose=True)
        nc.vector.tensor_copy(qselT2[:, qs], ps_qsT[:])
        ps_cbT = ps_m.tile([E, P], BF, tag="ps")
        nc.tensor.matmul(ps_cbT[:], comb8[:], ident_b[:], is_transpose=True)
        nc.vector.tensor_copy(combT[:, qs], ps_cbT[:])

    def emit_B(qt):
        emit_B_mm(qt)
        emit_B_fin(qt)

    # ---- D1 for one qt: transpose attn back, scale, scatter to cm ----
    cms = {}
    def emit_D1(qt):
        qs = slice(qt * P, (qt + 1) * P)
        h_at = []
        for h in range(2):
            ps_at = ps_m.tile([P, VW], BF, tag="ps")
            nc.tensor.matmul(ps_at[:], attnT[h][:, qs], ident_b[:VW, :VW],
                             is_transpose=True)
            at = dpool.tile([P, VW], FP, tag=f"at{qt % 2}_{h}",
                            name=f"at{qt % 2}_{h}")
            nc.vector.tensor_copy(at[:], ps_at[:])
            h_at.append(at)
        ds = dpool.tile([P, P], BF, tag=f"ds{qt % 2}", name=f"ds{qt % 2}")
        for h in range(2):
            # gpsimd: an = at / denom (frees the DVE of recip+mult)
            an = work.tile([P, DK], FP, tag="an")
            nc.gpsimd.normalize_recip(an[:], h_at[h][:, 0:DK],
                                      h_at[h][:, DK:DK + 1])
            nc.vector.tensor_scalar(ds[:, h * DK:(h + 1) * DK], an[:],
                                    g_all[:, 2 * qt + h:2 * qt + h + 1],
                                    None, op0=OP.mult)
        cm = dpool.tile([P, D], BF, tag=f"cm{qt % 2}", name=f"cm{qt % 2}")
        nc.gpsimd.local_scatter(cm[:], ds[:], idx16[:, qt * P:(qt + 1) * P],
                                channels=P, num_elems=D, num_idxs=P)
        cms[qt] = cm

    # ---- D3 for one qt: transpose cm, output projection, store ----
    def emit_D3(qt, late):
        qs = slice(qt * P, (qt + 1) * P)
        cm = cms.pop(qt)
        cTs = []
        tp = ps_log if late else ps_m
        for ci in range(DC):
            ps_ct = tp.tile([P, P], BF, tag="ps" if tp is ps_m else "ps_lg2",
                            name="ps_ct")
            nc.tensor.matmul(ps_ct[:], cm[:, ci * P:(ci + 1) * P], ident_b[:],
                             is_transpose=True)
            cT = work.tile([P, P], BF, tag=f"cT{ci}", name=f"cT{ci}")
            if ci % 2 == 0:
                nc.vector.tensor_copy(cT[:], ps_ct[:])
            else:
                nc.scalar.activation(cT[:], ps_ct[:], AF.Copy)
            cTs.append(cT)
        ps_o = ps_m.tile([P, D], FP, tag="ps")
        for ci in range(DC):
            nc.tensor.matmul(ps_o[:], cTs[ci][:], Wo_f[ci][:],
                             start=(ci == 0), stop=False)
        nc.tensor.matmul(ps_o[:], combT[:, qs], bo8, start=False, stop=True)
        o = work.tile([P, D], BF, tag="o")
        nc.scalar.activation(o[:], ps_o[:], AF.Copy)
        nc.sync.dma_start(out_d[qs, :], o[:])

    # ---- phase C for one half, with interleaved extra blocks ----
    def emit_C(half, extras):
        hs = slice(half * HD, (half + 1) * HD)
        ps_a = [ps_att.tile([VW, HD], FP, tag="ps_a", name=f"ps_a{h}")
                for h in range(2)]
        pending = None
        for kc in range(NKC):
            ps_lg2 = ps_log.tile([P, 2 * HD], FP, tag="ps_lg2")
            for h in range(2):
                rg = slice(h * DK, (h + 1) * DK)
                nc.tensor.matmul(ps_lg2[:, h * HD:(h + 1) * HD],
                                 khT2[rg, kc * P:(kc + 1) * P],
                                 qselT2[rg, hs], start=True, stop=True)
            if pending is not None:
                pkc, pex = pending
                for h in range(2):
                    nc.tensor.matmul(ps_a[h][:],
                                     vh_aug[:, pkc * VW:(pkc + 1) * VW],
                                     pex[:, h * HD:(h + 1) * HD],
                                     start=(pkc == 0), stop=(pkc == NKC - 1),
                                     skip_group_check=True)
            ex = expp.tile([P, 2 * HD], BF, tag="ex")
            nc.scalar.activation(ex[:], ps_lg2[:], AF.Exp)
            pending = (kc, ex)
            if kc in extras:
                extras[kc]()
        pkc, pex = pending
        for h in range(2):
            nc.tensor.matmul(ps_a[h][:], vh_aug[:, pkc * VW:(pkc + 1) * VW],
                             pex[:, h * HD:(h + 1) * HD],
                             start=(pkc == 0), stop=(pkc == NKC - 1),
                             skip_group_check=True)
        for h in range(2):
            nc.vector.tensor_copy(attnT[h][:, hs], ps_a[h][:])

    # ================= emission =================
    # B blocks for qts 0-3 (transposes deferred one block), vh interleaved
    for qt in range(4):
        emit_B_mm(qt)
        if qt > 0:
            emit_B_fin(qt - 1)
        for kc in range(4 * qt, 4 * qt + 4):
            emit_vh_kc(kc)
    emit_B_fin(3)

    # C half 0, with B(4..7) interleaved into the matmul stream
    emit_C(0, {1: lambda: emit_B_mm(4), 3: lambda: emit_B_fin(4),
               5: lambda: emit_B_mm(5), 7: lambda: emit_B_fin(5),
               9: lambda: emit_B_mm(6), 11: lambda: emit_B_fin(6),
               13: lambda: emit_B_mm(7), 15: lambda: emit_B_fin(7)})

    # C half 1, with D(0..2) interleaved (their attnT half-0 data is ready)
    emit_C(1, {1: lambda: emit_D1(0), 3: lambda: emit_D1(1),
               5: lambda: emit_D3(0, False), 7: lambda: emit_D1(2),
               9: lambda: emit_D3(1, False), 11: lambda: emit_D1(3),
               13: lambda: emit_D3(2, False)})

    # drain: remaining D, staggered so scatters run ahead of D3 matmuls
    emit_D1(4)
    emit_D1(5)
    emit_D3(3, True)
    emit_D1(6)
    emit_D3(4, True)
    emit_D1(7)
    emit_D3(5, True)
    emit_D3(6, True)
    emit_D3(7, True)


_PROGRAM = None


def get_program():
    global _PROGRAM
    if _PROGRAM is None:
        nc = bacc.Bacc("TRN2", target_bir_lowering=False, debug=False,
                       enable_asserts=False, num_devices=8)
        from contextlib import ExitStack
        with tile.TileContext(nc) as tc, ExitStack() as ctx:
            _emit(nc, tc, ctx)
        nc.compile()
        _PROGRAM = nc
    return _PROGRAM


def make_in_maps(query, key, value, Wg, Wk, bk, Wv, bv, Wq, bq, Wo, bo):
    import ml_dtypes
    BFNP = ml_dtypes.bfloat16

    def hilo(x):
        x = np.asarray(x, np.float32)
        hi = x.astype(BFNP)
        lo = (x - hi.astype(np.float32)).astype(BFNP)
        return hi, lo

    Wg_hi, Wg_lo = hilo(Wg)
    # kh pre-scaled by 1/8 == 1/sqrt(DK); doubled for the two head row-groups
    Wk2s = np.concatenate([np.asarray(Wk), np.asarray(Wk)], axis=1) * 0.125
    # bk shifts all logits of a query equally -> softmax-invariant: dropped.
    # (d e) ordering (col/row index = d*E + e)
    Wq_f = np.asarray(Wq).transpose(1, 2, 0).reshape(D, DK * E)
    Wo_f = np.asarray(Wo).transpose(1, 0, 2).reshape(DK * E, D)
    bq_f = np.asarray(bq).T.reshape(DK * E)
    # bv folds into bo since attention weights sum to 1
    bo_p = np.asarray(bo) + np.einsum('d,edm->em', np.asarray(bv, np.float32),
                                      np.asarray(Wo, np.float32))

    w0a = np.zeros((P, _W0A_COLS), BFNP)
    def put(name, rows, arr):
        base = _W0A[name]
        arr = np.asarray(arr, BFNP)
        w0a[rows, base:base + arr.shape[-1]] = arr
    for dc in range(DC):
        rows = slice(0, P)
        w0a[:, _W0A["Wk2s"] + dc * P:_W0A["Wk2s"] + (dc + 1) * P] = \
            np.asarray(Wk2s[dc * P:(dc + 1) * P, :], BFNP)
        w0a[:, _W0A["Wv"] + dc * DK:_W0A["Wv"] + (dc + 1) * DK] = \
            np.asarray(Wv, np.float32)[dc * P:(dc + 1) * P, :].astype(BFNP)
        w0a[:, _W0A["Wg_hi"] + dc * E:_W0A["Wg_hi"] + (dc + 1) * E] = \
            Wg_hi[dc * P:(dc + 1) * P, :]
        w0a[:, _W0A["Wg_lo"] + dc * E:_W0A["Wg_lo"] + (dc + 1) * E] = \
            Wg_lo[dc * P:(dc + 1) * P, :]
    w0a[:, _W0A["ones"]:_W0A["ones"] + P] = np.ones((P, P), BFNP)
    put("bq_row", 0, bq_f)
    put("bo", slice(0, E), bo_p)

    w0b = np.zeros((P, _W0B_COLS), BFNP)
    for dc in range(DC):
        w0b[:, dc * D:(dc + 1) * D] = \
            np.asarray(Wq_f, np.float32)[dc * P:(dc + 1) * P, :].astype(BFNP)
        w0b[:, DC * D + dc * D:DC * D + (dc + 1) * D] = \
            np.asarray(Wo_f, np.float32)[dc * P:(dc + 1) * P, :].astype(BFNP)

    def pack_chunks(x, n):  # [512, N] -> [128, 4*N] (dc-major columns)
        out = np.empty((P, DC * n), x.dtype)
        for dc in range(DC):
            out[:, dc * n:(dc + 1) * n] = x[dc * P:(dc + 1) * P, :]
        return np.ascontiguousarray(out)

    def pack_blocks(x, nblk, blkcols):  # [512, N] -> [128, nblk*(4*blkcols)]
        # block j = [dc0 cols | dc1 cols | dc2 cols | dc3 cols]
        out = np.empty((P, DC * nblk * blkcols), x.dtype)
        for j in range(nblk):
            for dc in range(DC):
                off = j * DC * blkcols + dc * blkcols
                out[:, off:off + blkcols] = \
                    x[dc * P:(dc + 1) * P, j * blkcols:(j + 1) * blkcols]
        return np.ascontiguousarray(out)

    shared = {"w0a": np.ascontiguousarray(w0a), "w0b": np.ascontiguousarray(w0b)}
    in_maps = []
    for b in range(4):
        kT = np.asarray(key[b], np.float32).T.astype(BFNP)     # [512, 2048]
        vT = np.asarray(value[b], np.float32).T.astype(BFNP)
        kTn = pack_blocks(kT, 4, HD)      # ncH-major blocks of 512 keys
        vTn = pack_blocks(vT, NKC, P)     # kc-major blocks of 128 keys
        for h in range(2):
            qhi, qlo = hilo(np.asarray(query[b][h * NQ:(h + 1) * NQ, :]).T)
            in_maps.append({"kTn": kTn, "vTn": vTn,
                            "qhi": pack_chunks(qhi, NQ),
                            "qlo": pack_chunks(qlo, NQ), **shared})
    return in_maps


def kernel(query, key, value, Wg, Wk, bk, Wv, bv, Wq, bq, Wo, bo):
    in_maps = make_in_maps(query, key, value, Wg, Wk, bk, Wv, bv, Wq, bq, Wo, bo)
    nc = get_program()
    res = bass_utils.run_bass_kernel_spmd(nc, in_maps, core_ids=list(range(8)))
    outs = [np.asarray(res.results[c]["out"], np.float32) for c in range(8)]
    return np.concatenate(outs, axis=0).reshape(4, T, D)


# revision 16
# speedup vs baseline: 1.5377x; 1.5377x over previous
"""MoE-routed multi-head attention kernel for 8 Trainium2 NeuronCores.

Problem shape (hardcoded):
  query/key/value: [4, 2048, 512] f32
  Wg [512,8], Wk/Wv [512,64], Wq [8,512,64], Wo [8,64,512], biases.
  TOP_K=2 routed experts act as the two attention heads.

Sharding: core c = 2*b + h handles batch b, query-half h (1024 query tokens),
with the full 2048 keys/values of batch b.

Key structural choices (v2):
  - bk is dropped: adding bk to kh shifts every logit of a query by a
    constant, which softmax ignores.  bv is folded into bo on the host
    (bo' = bo + bv @ Wo_e) since attn rows sum to 1 before the gate.
  - vh is computed directly key-major (vT chunks stationary, Wv moving),
    so no PE transposes of vh are needed.
  - Router logits (3-term bf16 hi/lo, fp32-exact for top-2 selection) are
    computed T-layout with tiny Wg-stationary loads, transposed per qt.
  - The expert gather (qsel from q_all) is a chain of 8 fused
    scalar_tensor_tensor ops per head; the expert scatter (attn -> cm)
    is a gpsimd local_scatter with per-token int16 indices.
  - Phase C: per kc one [128,1024] fp32 2-bank PSUM logits tile (both
    heads via row-grouped concurrent matmuls), one [128,1024] exp, and
    2 attention matmuls, software-pipelined.  B-blocks for qts 4-7 are
    interleaved into C(half0)'s PE stream, D-blocks into C(half1)'s, so
    no engine idles and the PE never sees a matmul-free window (HAM).
  - Output is written bf16; host casts to fp32.
"""

import numpy as np

import concourse.bass as bass
import concourse.mybir as mybir
import concourse.tile as tile
from concourse import bacc
from concourse import bass_utils
from concourse.masks import make_identity

P = 128
D = 512          # d_model
T = 2048         # kv tokens per core (full batch)
NQ = 1024        # query tokens per core
E = 8            # experts
DK = 64          # head dim
DC = D // P      # 4 contraction chunks
NKC = T // P     # 16 key chunks
NQT = NQ // P    # 8 query tiles
VW = DK + 1      # vh columns + ones column (denominator trick)
HD = 512         # phase-C column granularity (half of NQ)

FP = mybir.dt.float32
U32 = mybir.dt.uint32
I16 = mybir.dt.int16
BF = mybir.dt.bfloat16
AF = mybir.ActivationFunctionType
OP = mybir.AluOpType
AX = mybir.AxisListType

USE_GPSIMD_SCATTER = True
ACT_ACCUM = True         # activation accum_out for softmax row sums

# ---- w0a packed layout (bf16 columns) ----
_W0A = {}
_off = 0
for name, cols in [("Wk2s", DC * P), ("Wv", DC * DK), ("Wg_hi", DC * E),
                   ("Wg_lo", DC * E), ("ones", P), ("bq_row", D), ("bo", D)]:
    _W0A[name] = _off
    _off += cols
_W0A_COLS = _off
_W0B_COLS = 2 * DC * D   # Wq_f (e d) | Wo_f (e d)


def _emit(nc, tc, ctx):
    const = ctx.enter_context(tc.tile_pool(name="const", bufs=1))
    persist = ctx.enter_context(tc.tile_pool(name="persist", bufs=1))
    work = ctx.enter_context(tc.tile_pool(name="work", bufs=3))
    expp = ctx.enter_context(tc.tile_pool(name="expp", bufs=3))
    dpool = ctx.enter_context(tc.tile_pool(name="dpool", bufs=2))
    ps_log = ctx.enter_context(tc.tile_pool(name="ps_log", bufs=2, space="PSUM"))
    ps_att = ctx.enter_context(tc.tile_pool(name="ps_att", bufs=2, space="PSUM"))
    ps_m = ctx.enter_context(tc.tile_pool(name="ps_m", bufs=2, space="PSUM"))

    dram = {}
    for name, shape, dt in [
        ("w0a", [P, _W0A_COLS], BF), ("w0b", [P, _W0B_COLS], BF),
        ("kTn", [P, DC * T], BF), ("vTn", [P, DC * T], BF),
        ("qhi", [P, DC * NQ], BF), ("qlo", [P, DC * NQ], BF),
    ]:
        dram[name] = nc.dram_tensor(name, shape, dt, kind="ExternalInput").ap()
    out_d = nc.dram_tensor("out", [NQ, D], BF, kind="ExternalOutput").ap()

    # ---- DMAs, ordered by first use; kT split so khT2 starts early ----
    w0a = const.tile([P, _W0A_COLS], BF, tag="w0a")
    nc.sync.dma_start(w0a[:], dram["w0a"])
    kTt = persist.tile([P, DC * T], BF, tag="kTt")
    for j in range(4):
        cs = slice(j * T, (j + 1) * T)
        nc.sync.dma_start(kTt[:, cs], dram["kTn"][:, cs])
    qhi_t = persist.tile([P, DC * NQ], BF, tag="qhi")
    nc.sync.dma_start(qhi_t[:], dram["qhi"])
    qlo_t = persist.tile([P, DC * NQ], BF, tag="qlo")
    nc.sync.dma_start(qlo_t[:], dram["qlo"])
    w0b = const.tile([P, _W0B_COLS], BF, tag="w0b")
    nc.sync.dma_start(w0b[:], dram["w0b"])
    vTt = persist.tile([P, DC * T], BF, tag="vTt")
    for j in range(4):
        cs = slice(j * T, (j + 1) * T)
        nc.sync.dma_start(vTt[:, cs], dram["vTn"][:, cs])

    def w0(name, r0, r1, c0, c1):
        base = _W0A[name]
        return w0a[r0:r1, base + c0:base + c1]

    Wk2s = {dc: w0("Wk2s", 0, P, dc * P, (dc + 1) * P) for dc in range(DC)}
    Wv = {dc: w0("Wv", 0, P, dc * DK, (dc + 1) * DK) for dc in range(DC)}
    Wg_hi = {dc: w0("Wg_hi", 0, P, dc * E, (dc + 1) * E) for dc in range(DC)}
    Wg_lo = {dc: w0("Wg_lo", 0, P, dc * E, (dc + 1) * E) for dc in range(DC)}
    ones_blk = w0("ones", 0, P, 0, P)
    ones_row = w0("ones", 0, 1, 0, P)        # [1,128] lhsT for bias matmul
    bq_row = w0("bq_row", 0, 1, 0, D)        # [1,512] (e d) order
    bo8 = w0("bo", 0, E, 0, D)               # [8,512] bo' = bo + bv@Wo
    Wq_f = {dc: w0b[:, dc * D:(dc + 1) * D] for dc in range(DC)}
    Wo_f = {dc: w0b[:, DC * D + dc * D:DC * D + (dc + 1) * D] for dc in range(DC)}
    qT_hi = {dc: qhi_t[:, dc * NQ:(dc + 1) * NQ] for dc in range(DC)}
    qT_lo = {dc: qlo_t[:, dc * NQ:(dc + 1) * NQ] for dc in range(DC)}

    # ---- constants ----
    ident = const.tile([P, P], FP, tag="ident")
    make_identity(nc, ident[:])
    ident_b = const.tile([P, P], BF, tag="ident_b")
    make_identity(nc, ident_b[:])
    iota8 = const.tile([P, E], FP, tag="iota8")
    nc.gpsimd.iota(iota8[:], pattern=[[1, E]], channel_multiplier=0,
                   allow_small_or_imprecise_dtypes=True)
    iota64x8 = const.tile([P, DK], FP, tag="iota64x8")   # 0,8,16,...,504
    nc.gpsimd.iota(iota64x8[:], pattern=[[8, DK]], channel_multiplier=0,
                   allow_small_or_imprecise_dtypes=True)
    iota_e = const.tile([P, DK * E], BF, tag="iota_e")   # expert id, (d e) cols
    nc.gpsimd.iota(iota_e[:].rearrange("p (d e) -> p d e", e=E),
                   pattern=[[0, DK], [1, E]], channel_multiplier=0,
                   allow_small_or_imprecise_dtypes=True)

    # ---- persistent intermediates ----
    khT2 = persist.tile([P, T], BF, tag="khT2")
    vh_aug = persist.tile([P, NKC * VW], BF, tag="vh_aug")
    routerT = persist.tile([E, NQ], FP, tag="routerT")
    qselT2 = persist.tile([P, NQ], BF, tag="qselT2")
    combT = persist.tile([E, NQ], BF, tag="combT")
    attnT = [persist.tile([VW, NQ], BF, tag=f"attnT{h}", name=f"attnT{h}")
             for h in range(2)]
    g_all = persist.tile([P, 2 * NQT], FP, tag="g_all")    # gates, 2 per qt
    idx16 = persist.tile([P, P * NQT], I16, tag="idx16")   # scatter idxs per qt

    # vh_aug ones columns (col 64 of each kc block)
    nc.vector.tensor_copy(
        vh_aug[:].rearrange("p (c w) -> p c w", w=VW)[:, :, DK],
        ones_blk[:, 0:NKC])

    # ---- A1: khT2 [128, T] (head-doubled kh, pre-scaled by 1/8) ----
    def emit_khT2_chunk(j):
        cs = slice(j * HD, (j + 1) * HD)
        ps = ps_m.tile([P, HD], FP, tag="ps")
        for dc in range(DC):
            nc.tensor.matmul(ps[:], Wk2s[dc][:], kTt[:, j * T + dc * HD:
                                                     j * T + (dc + 1) * HD],
                             start=(dc == 0), stop=(dc == DC - 1))
        nc.vector.tensor_copy(khT2[:, cs], ps[:])

    for j in range(4):
        emit_khT2_chunk(j)

    # ---- A2: router logits, T-layout, 3-term hi/lo (fp32-exact) ----
    for half in range(2):
        hs = slice(half * HD, (half + 1) * HD)
        ps_r = ps_m.tile([E, HD], FP, tag="ps")
        first = True
        for dc in range(DC):
            for wg, q in [(Wg_hi, qT_hi), (Wg_lo, qT_hi), (Wg_hi, qT_lo)]:
                nc.tensor.matmul(ps_r[:], wg[dc][:], q[dc][:, hs],
                                 start=first,
                                 stop=(dc == DC - 1 and q is qT_lo))
                first = False
        nc.vector.tensor_copy(routerT[:, hs], ps_r[:])

    # ---- A3 (interleaved into B blocks): vh key-major -> vh_aug ----
    def emit_vh_kc(kc):
        # shares the ps_a tag: vh use ends before phase C allocates ps_a
        ps = ps_att.tile([P, DK], FP, tag="ps_a", name="ps_vh")
        for dc in range(DC):
            nc.tensor.matmul(ps[:], vTt[:, kc * D + dc * P:kc * D + (dc + 1) * P],
                             Wv[dc][:], start=(dc == 0), stop=(dc == DC - 1))
        nc.vector.tensor_copy(vh_aug[:, kc * VW:kc * VW + DK], ps[:])

    # ---- B block for one qt: q_all, top-2 select, gather, gates ----
    # Split in two: the PE/ACT/DVE producer part, and a deferred finish
    # (the qsel2/comb8 transposes) emitted one interleave slot later so
    # the PE never waits on the DVE gather.
    b_state = {}

    def emit_B_mm(qt):
        qs = slice(qt * P, (qt + 1) * P)
        # PE: all-expert query projection + bq bias row
        ps_qa = ps_m.tile([P, D], FP, tag="ps")
        for dc in range(DC):
            nc.tensor.matmul(ps_qa[:], qT_hi[dc][:, qs], Wq_f[dc][:],
                             start=(dc == 0), stop=False)
        nc.tensor.matmul(ps_qa[:], ones_row, bq_row, start=False, stop=True)
        qa_b = work.tile([P, D], BF, tag="qa_b")
        nc.scalar.activation(qa_b[:], ps_qa[:], AF.Copy)
        # PE: router logits transpose for this qt
        ps_lg = ps_m.tile([P, E], FP, tag="ps")
        nc.tensor.matmul(ps_lg[:], routerT[:, qs], ident[:E, :E],
                         is_transpose=True)
        lg8 = work.tile([P, E], FP, tag="lg8")
        nc.vector.tensor_copy(lg8[:], ps_lg[:])
        # top-2 selection
        m8 = work.tile([P, E], FP, tag="m8")
        nc.vector.max(out=m8[:], in_=lg8[:])
        i8 = work.tile([P, E], U32, tag="i8")
        nc.vector.max_index(i8[:], m8[:], lg8[:])
        if8 = work.tile([P, 2], FP, tag="if8")
        nc.vector.tensor_copy(if8[:], i8[:, 0:2])
        # softmax pieces: e8 + row sum, top-2 exps
        e8 = work.tile([P, E], FP, tag="e8")
        ssum = work.tile([P, 1], FP, tag="ssum")
        if ACT_ACCUM:
            nc.scalar.activation(e8[:], lg8[:], AF.Exp, accum_out=ssum[:])
        else:
            nc.scalar.activation(e8[:], lg8[:], AF.Exp)
            nc.vector.reduce_sum(ssum[:], e8[:], axis=AX.X)
        gtop = work.tile([P, 2], FP, tag="gtop")
        nc.scalar.activation(gtop[:], m8[:, 0:2], AF.Exp)
        srec = work.tile([P, 1], FP, tag="srec")
        nc.vector.reciprocal(srec[:], ssum[:])
        g = g_all[:, 2 * qt:2 * qt + 2]
        nc.vector.tensor_scalar(g, gtop[:], srec[:], None, op0=OP.mult)
        # masked gather: one fused (iota==e_h)*qa per head, then e-reduce
        qsel2 = work.tile([P, P], BF, tag="qsel2")
        for h in range(2):
            u = work.tile([P, D], BF, tag=f"u{h}", name=f"u{h}")
            nc.vector.scalar_tensor_tensor(
                u[:], iota_e[:], if8[:, h:h + 1], qa_b[:],
                op0=OP.is_equal, op1=OP.mult)
            with nc.allow_low_precision(reason="one-hot masked sum: only one "
                                        "of the 8 summands is nonzero"):
                nc.vector.reduce_sum(qsel2[:, h * DK:(h + 1) * DK],
                                     u[:].rearrange("p (d e) -> p d e", e=E),
                                     axis=AX.X)
        # scatter indices for phase D (int16, (d e) order: 8*d + e_h)
        if USE_GPSIMD_SCATTER:
            for h in range(2):
                nc.vector.tensor_scalar(
                    idx16[:, qt * P + h * DK:qt * P + (h + 1) * DK],
                    iota64x8[:], if8[:, h:h + 1], None, op0=OP.add)
        # gate-weighted one-hot combine row (for bo' bias matmul)
        comb8 = work.tile([P, E], BF, tag="comb8")
        tmp8 = work.tile([P, E], BF, tag="tmp8")
        nc.vector.scalar_tensor_tensor(
            comb8[:], iota8[:], if8[:, 0:1],
            g[:, 0:1].to_broadcast((P, E)), op0=OP.is_equal, op1=OP.mult)
        nc.vector.scalar_tensor_tensor(
            tmp8[:], iota8[:], if8[:, 1:2],
            g[:, 1:2].to_broadcast((P, E)), op0=OP.is_equal, op1=OP.mult)
        nc.vector.tensor_tensor(comb8[:], comb8[:], tmp8[:], op=OP.add)
        b_state[qt] = (qsel2, comb8)

    def emit_B_fin(qt):
        qs = slice(qt * P, (qt + 1) * P)
        qsel2, comb8 = b_state.pop(qt)
        ps_qsT = ps_m.tile([P, P], BF, tag="ps")
        nc.tensor.matmul(ps_qsT[:], qsel2[:], ident_b[:], is_transpose=True)
        nc.vector.tensor_copy(qselT2[:, qs], ps_qsT[:])
        ps_cbT = ps_m.tile([E, P], BF, tag="ps")
        nc.tensor.matmul(ps_cbT[:], comb8[:], ident_b[:], is_transpose=True)
        nc.vector.tensor_copy(combT[:, qs], ps_cbT[:])

    def emit_B(qt):
        emit_B_mm(qt)
        emit_B_fin(qt)

    # ---- D1 for one qt: transpose attn back, scale, scatter to cm ----
    cms = {}
    def emit_D1(qt):
        qs = slice(qt * P, (qt + 1) * P)
        h_at = []
        for h in range(2):
            ps_at = ps_m.tile([P, VW], BF, tag="ps")
            nc.tensor.matmul(ps_at[:], attnT[h][:, qs], ident_b[:VW, :VW],
                             is_transpose=True)
            at = dpool.tile([P, VW], FP, tag=f"at{qt % 2}_{h}",
                            name=f"at{qt % 2}_{h}")
            nc.vector.tensor_copy(at[:], ps_at[:])
            h_at.append(at)
        ds = dpool.tile([P, P], BF, tag=f"ds{qt % 2}", name=f"ds{qt % 2}")
        for h in range(2):
            # s = gate / denom, then ds = at * s (keep gpsimd scatter-only:
            # mixing gpsimd libraries forces ~2us IRAM reloads per call)
            dinv = work.tile([P, 1], FP, tag="dinv")
            nc.vector.reciprocal(dinv[:], h_at[h][:, DK:DK + 1])
            s = work.tile([P, 1], FP, tag="s")
            nc.vector.tensor_tensor(s[:], g_all[:, 2 * qt + h:2 * qt + h + 1],
                                    dinv[:], op=OP.mult)
            nc.vector.tensor_scalar(ds[:, h * DK:(h + 1) * DK],
                                    h_at[h][:, 0:DK], s[:], None, op0=OP.mult)
        cm = dpool.tile([P, D], BF, tag=f"cm{qt % 2}", name=f"cm{qt % 2}")
        nc.gpsimd.local_scatter(cm[:], ds[:], idx16[:, qt * P:(qt + 1) * P],
                                channels=P, num_elems=D, num_idxs=P)
        cms[qt] = cm

    # ---- D3 for one qt: transpose cm, output projection, store ----
    def emit_D3(qt, late):
        qs = slice(qt * P, (qt + 1) * P)
        cm = cms.pop(qt)
        cTs = []
        tp = ps_log if late else ps_m
        for ci in range(DC):
            ps_ct = tp.tile([P, P], BF, tag="ps" if tp is ps_m else "ps_lg2",
                            name="ps_ct")
            nc.tensor.matmul(ps_ct[:], cm[:, ci * P:(ci + 1) * P], ident_b[:],
                             is_transpose=True)
            cT = work.tile([P, P], BF, tag=f"cT{ci}", name=f"cT{ci}")
            if ci % 2 == 0:
                nc.vector.tensor_copy(cT[:], ps_ct[:])
            else:
                nc.scalar.activation(cT[:], ps_ct[:], AF.Copy)
            cTs.append(cT)
        ps_o = ps_m.tile([P, D], FP, tag="ps")
        for ci in range(DC):
            nc.tensor.matmul(ps_o[:], cTs[ci][:], Wo_f[ci][:],
                             start=(ci == 0), stop=False)
        nc.tensor.matmul(ps_o[:], combT[:, qs], bo8, start=False, stop=True)
        o = work.tile([P, D], BF, tag="o")
        nc.scalar.activation(o[:], ps_o[:], AF.Copy)
        nc.sync.dma_start(out_d[qs, :], o[:])

    # ---- phase C for one half, with interleaved extra blocks ----
    def emit_C(half, extras):
        hs = slice(half * HD, (half + 1) * HD)
        ps_a = [ps_att.tile([VW, HD], FP, tag="ps_a", name=f"ps_a{h}")
                for h in range(2)]
        pending = None
        for kc in range(NKC):
            ps_lg2 = ps_log.tile([P, 2 * HD], FP, tag="ps_lg2")
            for h in range(2):
                rg = slice(h * DK, (h + 1) * DK)
                nc.tensor.matmul(ps_lg2[:, h * HD:(h + 1) * HD],
                                 khT2[rg, kc * P:(kc + 1) * P],
                                 qselT2[rg, hs], start=True, stop=True)
            if pending is not None:
                pkc, pex = pending
                for h in range(2):
                    nc.tensor.matmul(ps_a[h][:],
                                     vh_aug[:, pkc * VW:(pkc + 1) * VW],
                                     pex[:, h * HD:(h + 1) * HD],
                                     start=(pkc == 0), stop=(pkc == NKC - 1),
                                     skip_group_check=True)
            ex = expp.tile([P, 2 * HD], BF, tag="ex")
            nc.scalar.activation(ex[:], ps_lg2[:], AF.Exp)
            pending = (kc, ex)
            if kc in extras:
                extras[kc]()
        pkc, pex = pending
        for h in range(2):
            nc.tensor.matmul(ps_a[h][:], vh_aug[:, pkc * VW:(pkc + 1) * VW],
                             pex[:, h * HD:(h + 1) * HD],
                             start=(pkc == 0), stop=(pkc == NKC - 1),
                             skip_group_check=True)
        for h in range(2):
            nc.vector.tensor_copy(attnT[h][:, hs], ps_a[h][:])

    # ================= emission =================
    # B blocks for qts 0-3 (transposes deferred one block), vh interleaved
    for qt in range(4):
        emit_B_mm(qt)
        if qt > 0:
            emit_B_fin(qt - 1)
        for kc in range(4 * qt, 4 * qt + 4):
            emit_vh_kc(kc)
    emit_B_fin(3)

    # C half 0, with B(4..7) interleaved into the matmul stream
    emit_C(0, {1: lambda: emit_B_mm(4), 3: lambda: emit_B_fin(4),
               5: lambda: emit_B_mm(5), 7: lambda: emit_B_fin(5),
               9: lambda: emit_B_mm(6), 11: lambda: emit_B_fin(6),
               13: lambda: emit_B_mm(7), 15: lambda: emit_B_fin(7)})

    # C half 1, with D(0..2) interleaved (their attnT half-0 data is ready)
    emit_C(1, {1: lambda: emit_D1(0), 3: lambda: emit_D1(1),
               5: lambda: emit_D3(0, False), 7: lambda: emit_D1(2),
               9: lambda: emit_D3(1, False), 11: lambda: emit_D1(3),
               13: lambda: emit_D3(2, False)})

    # drain: remaining D, staggered so scatters run ahead of D3 matmuls
    emit_D1(4)
    emit_D1(5)
    emit_D3(3, True)
    emit_D1(6)
    emit_D3(4, True)
    emit_D1(7)
    emit_D3(5, True)
    emit_D3(6, True)
    emit_D3(7, True)


_PROGRAM = None


def get_program():
    global _PROGRAM
    if _PROGRAM is None:
        nc = bacc.Bacc("TRN2", target_bir_lowering=False, debug=False,
                       enable_asserts=False, num_devices=8)
        from contextlib import ExitStack
        with tile.TileContext(nc) as tc, ExitStack() as ctx:
            _emit(nc, tc, ctx)
        nc.compile()
        _PROGRAM = nc
    return _PROGRAM


def make_in_maps(query, key, value, Wg, Wk, bk, Wv, bv, Wq, bq, Wo, bo):
    import ml_dtypes
    BFNP = ml_dtypes.bfloat16

    def hilo(x):
        x = np.asarray(x, np.float32)
        hi = x.astype(BFNP)
        lo = (x - hi.astype(np.float32)).astype(BFNP)
        return hi, lo

    Wg_hi, Wg_lo = hilo(Wg)
    # kh pre-scaled by 1/8 == 1/sqrt(DK); doubled for the two head row-groups
    Wk2s = np.concatenate([np.asarray(Wk), np.asarray(Wk)], axis=1) * 0.125
    # bk shifts all logits of a query equally -> softmax-invariant: dropped.
    # (d e) ordering (col/row index = d*E + e)
    Wq_f = np.asarray(Wq).transpose(1, 2, 0).reshape(D, DK * E)
    Wo_f = np.asarray(Wo).transpose(1, 0, 2).reshape(DK * E, D)
    bq_f = np.asarray(bq).T.reshape(DK * E)
    # bv folds into bo since attention weights sum to 1
    bo_p = np.asarray(bo) + np.einsum('d,edm->em', np.asarray(bv, np.float32),
                                      np.asarray(Wo, np.float32))

    w0a = np.zeros((P, _W0A_COLS), BFNP)
    def put(name, rows, arr):
        base = _W0A[name]
        arr = np.asarray(arr, BFNP)
        w0a[rows, base:base + arr.shape[-1]] = arr
    for dc in range(DC):
        rows = slice(0, P)
        w0a[:, _W0A["Wk2s"] + dc * P:_W0A["Wk2s"] + (dc + 1) * P] = \
            np.asarray(Wk2s[dc * P:(dc + 1) * P, :], BFNP)
        w0a[:, _W0A["Wv"] + dc * DK:_W0A["Wv"] + (dc + 1) * DK] = \
            np.asarray(Wv, np.float32)[dc * P:(dc + 1) * P, :].astype(BFNP)
        w0a[:, _W0A["Wg_hi"] + dc * E:_W0A["Wg_hi"] + (dc + 1) * E] = \
            Wg_hi[dc * P:(dc + 1) * P, :]
        w0a[:, _W0A["Wg_lo"] + dc * E:_W0A["Wg_lo"] + (dc + 1) * E] = \
            Wg_lo[dc * P:(dc + 1) * P, :]
    w0a[:, _W0A["ones"]:_W0A["ones"] + P] = np.ones((P, P), BFNP)
    put("bq_row", 0, bq_f)
    put("bo", slice(0, E), bo_p)

    w0b = np.zeros((P, _W0B_COLS), BFNP)
    for dc in range(DC):
        w0b[:, dc * D:(dc + 1) * D] = \
            np.asarray(Wq_f, np.float32)[dc * P:(dc + 1) * P, :].astype(BFNP)
        w0b[:, DC * D + dc * D:DC * D + (dc + 1) * D] = \
            np.asarray(Wo_f, np.float32)[dc * P:(dc + 1) * P, :].astype(BFNP)

    def pack_chunks(x, n):  # [512, N] -> [128, 4*N] (dc-major columns)
        out = np.empty((P, DC * n), x.dtype)
        for dc in range(DC):
            out[:, dc * n:(dc + 1) * n] = x[dc * P:(dc + 1) * P, :]
        return np.ascontiguousarray(out)

    def pack_blocks(x, nblk, blkcols):  # [512, N] -> [128, nblk*(4*blkcols)]
        # block j = [dc0 cols | dc1 cols | dc2 cols | dc3 cols]
        out = np.empty((P, DC * nblk * blkcols), x.dtype)
        for j in range(nblk):
            for dc in range(DC):
                off = j * DC * blkcols + dc * blkcols
                out[:, off:off + blkcols] = \
                    x[dc * P:(dc + 1) * P, j * blkcols:(j + 1) * blkcols]
        return np.ascontiguousarray(out)

    shared = {"w0a": np.ascontiguousarray(w0a), "w0b": np.ascontiguousarray(w0b)}
    in_maps = []
    for b in range(4):
        kT = np.asarray(key[b], np.float32).T.astype(BFNP)     # [512, 2048]
        vT = np.asarray(value[b], np.float32).T.astype(BFNP)
        kTn = pack_blocks(kT, 4, HD)      # ncH-major blocks of 512 keys
        vTn = pack_blocks(vT, NKC, P)     # kc-major blocks of 128 keys
        for h in range(2):
            qhi, qlo = hilo(np.asarray(query[b][h * NQ:(h + 1) * NQ, :]).T)
            in_maps.append({"kTn": kTn, "vTn": vTn,
                            "qhi": pack_chunks(qhi, NQ),
                            "qlo": pack_chunks(qlo, NQ), **shared})
    return in_maps


def kernel(query, key, value, Wg, Wk, bk, Wv, bv, Wq, bq, Wo, bo):
    in_maps = make_in_maps(query, key, value, Wg, Wk, bk, Wv, bv, Wq, bq, Wo, bo)
    nc = get_program()
    res = bass_utils.run_bass_kernel_spmd(nc, in_maps, core_ids=list(range(8)))
    outs = [np.asarray(res.results[c]["out"], np.float32) for c in range(8)]
    return np.concatenate(outs, axis=0).reshape(4, T, D)


# revision 20
# speedup vs baseline: 1.5538x; 1.0105x over previous
"""MoE-routed multi-head attention kernel for 8 Trainium2 NeuronCores.

Problem shape (hardcoded):
  query/key/value: [4, 2048, 512] f32
  Wg [512,8], Wk/Wv [512,64], Wq [8,512,64], Wo [8,64,512], biases.
  TOP_K=2 routed experts act as the two attention heads.

Sharding: core c = 2*b + h handles batch b, query-half h (1024 query tokens),
with the full 2048 keys/values of batch b.

Key structural choices (v2):
  - bk is dropped: adding bk to kh shifts every logit of a query by a
    constant, which softmax ignores.  bv is folded into bo on the host
    (bo' = bo + bv @ Wo_e) since attn rows sum to 1 before the gate.
  - vh is computed directly key-major (vT chunks stationary, Wv moving),
    so no PE transposes of vh are needed.
  - Router logits (3-term bf16 hi/lo, fp32-exact for top-2 selection) are
    computed T-layout with tiny Wg-stationary loads, transposed per qt.
  - The expert gather (qsel from q_all) is a chain of 8 fused
    scalar_tensor_tensor ops per head; the expert scatter (attn -> cm)
    is a gpsimd local_scatter with per-token int16 indices.
  - Phase C: per kc one [128,1024] fp32 2-bank PSUM logits tile (both
    heads via row-grouped concurrent matmuls), one [128,1024] exp, and
    2 attention matmuls, software-pipelined.  B-blocks for qts 4-7 are
    interleaved into C(half0)'s PE stream, D-blocks into C(half1)'s, so
    no engine idles and the PE never sees a matmul-free window (HAM).
  - Output is written bf16; host casts to fp32.
"""

import numpy as np

import concourse.bass as bass
import concourse.mybir as mybir
import concourse.tile as tile
from concourse import bacc
from concourse import bass_utils
from concourse.masks import make_identity

P = 128
D = 512          # d_model
T = 2048         # kv tokens per core (full batch)
NQ = 1024        # query tokens per core
E = 8            # experts
DK = 64          # head dim
DC = D // P      # 4 contraction chunks
NKC = T // P     # 16 key chunks
NQT = NQ // P    # 8 query tiles
VW = DK + 1      # vh columns + ones column (denominator trick)
HD = 512         # phase-C column granularity (half of NQ)

FP = mybir.dt.float32
U32 = mybir.dt.uint32
I16 = mybir.dt.int16
BF = mybir.dt.bfloat16
AF = mybir.ActivationFunctionType
OP = mybir.AluOpType
AX = mybir.AxisListType

USE_GPSIMD_SCATTER = True
ACT_ACCUM = True         # activation accum_out for softmax row sums

# ---- w0a packed layout (bf16 columns) ----
_W0A = {}
_off = 0
for name, cols in [("Wk2s", DC * P), ("Wv", DC * DK), ("Wg_hi", DC * E),
                   ("Wg_lo", DC * E), ("ones", P), ("bq_row", D), ("bo", D)]:
    _W0A[name] = _off
    _off += cols
_W0A_COLS = _off
_W0B_COLS = 2 * DC * D   # Wq_f (e d) | Wo_f (e d)


def _emit(nc, tc, ctx):
    const = ctx.enter_context(tc.tile_pool(name="const", bufs=1))
    persist = ctx.enter_context(tc.tile_pool(name="persist", bufs=1))
    work = ctx.enter_context(tc.tile_pool(name="work", bufs=3))
    expp = ctx.enter_context(tc.tile_pool(name="expp", bufs=3))
    dpool = ctx.enter_context(tc.tile_pool(name="dpool", bufs=2))
    ps_log = ctx.enter_context(tc.tile_pool(name="ps_log", bufs=2, space="PSUM"))
    ps_att = ctx.enter_context(tc.tile_pool(name="ps_att", bufs=2, space="PSUM"))
    ps_m = ctx.enter_context(tc.tile_pool(name="ps_m", bufs=2, space="PSUM"))

    dram = {}
    for name, shape, dt in [
        ("w0a", [P, _W0A_COLS], BF), ("w0b", [P, _W0B_COLS], BF),
        ("kTn", [P, DC * T], BF), ("vTn", [P, DC * T], BF),
        ("qhi", [P, DC * NQ], BF), ("qlo", [P, DC * NQ], BF),
    ]:
        dram[name] = nc.dram_tensor(name, shape, dt, kind="ExternalInput").ap()
    out_d = nc.dram_tensor("out", [NQ, D], BF, kind="ExternalOutput").ap()

    # ---- DMAs, ordered by first use; kT split so khT2 starts early ----
    w0a = const.tile([P, _W0A_COLS], BF, tag="w0a")
    nc.sync.dma_start(w0a[:, :DC * P], dram["w0a"][:, :DC * P])  # Wk2s first
    nc.sync.dma_start(w0a[:, DC * P:], dram["w0a"][:, DC * P:])
    kTt = persist.tile([P, DC * T], BF, tag="kTt")
    for j in range(4):
        cs = slice(j * T, (j + 1) * T)
        nc.sync.dma_start(kTt[:, cs], dram["kTn"][:, cs])
    qhi_t = persist.tile([P, DC * NQ], BF, tag="qhi")
    nc.sync.dma_start(qhi_t[:], dram["qhi"])
    qlo_t = persist.tile([P, DC * NQ], BF, tag="qlo")
    nc.sync.dma_start(qlo_t[:], dram["qlo"])
    w0b = const.tile([P, _W0B_COLS], BF, tag="w0b")
    nc.sync.dma_start(w0b[:], dram["w0b"])
    vTt = persist.tile([P, DC * T], BF, tag="vTt")
    for j in range(4):
        cs = slice(j * T, (j + 1) * T)
        nc.sync.dma_start(vTt[:, cs], dram["vTn"][:, cs])

    def w0(name, r0, r1, c0, c1):
        base = _W0A[name]
        return w0a[r0:r1, base + c0:base + c1]

    Wk2s = {dc: w0("Wk2s", 0, P, dc * P, (dc + 1) * P) for dc in range(DC)}
    Wv = {dc: w0("Wv", 0, P, dc * DK, (dc + 1) * DK) for dc in range(DC)}
    Wg_hi = {dc: w0("Wg_hi", 0, P, dc * E, (dc + 1) * E) for dc in range(DC)}
    Wg_lo = {dc: w0("Wg_lo", 0, P, dc * E, (dc + 1) * E) for dc in range(DC)}
    ones_blk = w0("ones", 0, P, 0, P)
    ones_row = w0("ones", 0, 1, 0, P)        # [1,128] lhsT for bias matmul
    bq_row = w0("bq_row", 0, 1, 0, D)        # [1,512] (e d) order
    bo8 = w0("bo", 0, E, 0, D)               # [8,512] bo' = bo + bv@Wo
    Wq_f = {dc: w0b[:, dc * D:(dc + 1) * D] for dc in range(DC)}
    Wo_f = {dc: w0b[:, DC * D + dc * D:DC * D + (dc + 1) * D] for dc in range(DC)}
    qT_hi = {dc: qhi_t[:, dc * NQ:(dc + 1) * NQ] for dc in range(DC)}
    qT_lo = {dc: qlo_t[:, dc * NQ:(dc + 1) * NQ] for dc in range(DC)}

    # ---- constants ----
    ident = const.tile([P, P], FP, tag="ident")
    make_identity(nc, ident[:])
    ident_b = const.tile([P, P], BF, tag="ident_b")
    make_identity(nc, ident_b[:])
    iota8 = const.tile([P, E], FP, tag="iota8")
    nc.gpsimd.iota(iota8[:], pattern=[[1, E]], channel_multiplier=0,
                   allow_small_or_imprecise_dtypes=True)
    iota64x8 = const.tile([P, DK], FP, tag="iota64x8")   # 0,8,16,...,504
    nc.gpsimd.iota(iota64x8[:], pattern=[[8, DK]], channel_multiplier=0,
                   allow_small_or_imprecise_dtypes=True)
    iota_e = const.tile([P, DK * E], BF, tag="iota_e")   # expert id, (d e) cols
    nc.gpsimd.iota(iota_e[:].rearrange("p (d e) -> p d e", e=E),
                   pattern=[[0, DK], [1, E]], channel_multiplier=0,
                   allow_small_or_imprecise_dtypes=True)

    # ---- persistent intermediates ----
    khT2 = persist.tile([P, T], BF, tag="khT2")
    vh_aug = persist.tile([P, NKC * VW], BF, tag="vh_aug")
    routerT = persist.tile([E, NQ], FP, tag="routerT")
    qselT2 = persist.tile([P, NQ], BF, tag="qselT2")
    combT = persist.tile([E, NQ], BF, tag="combT")
    attnT = [persist.tile([VW, NQ], BF, tag=f"attnT{h}", name=f"attnT{h}")
             for h in range(2)]
    g_all = persist.tile([P, 2 * NQT], FP, tag="g_all")    # gates, 2 per qt
    idx16 = persist.tile([P, P * NQT], I16, tag="idx16")   # scatter idxs per qt

    # vh_aug ones columns (col 64 of each kc block)
    nc.vector.tensor_copy(
        vh_aug[:].rearrange("p (c w) -> p c w", w=VW)[:, :, DK],
        ones_blk[:, 0:NKC])

    # ---- A1: khT2 [128, T] (head-doubled kh, pre-scaled by 1/8) ----
    def emit_khT2_chunk(j):
        cs = slice(j * HD, (j + 1) * HD)
        ps = ps_m.tile([P, HD], FP, tag="ps")
        for dc in range(DC):
            nc.tensor.matmul(ps[:], Wk2s[dc][:], kTt[:, j * T + dc * HD:
                                                     j * T + (dc + 1) * HD],
                             start=(dc == 0), stop=(dc == DC - 1))
        nc.scalar.activation(khT2[:, cs], ps[:], AF.Copy)

    for j in range(4):
        emit_khT2_chunk(j)

    # ---- A2: router logits, T-layout, 3-term hi/lo (fp32-exact) ----
    for half in range(2):
        hs = slice(half * HD, (half + 1) * HD)
        ps_r = ps_m.tile([E, HD], FP, tag="ps")
        terms = [(wg, dc, q) for q in (qT_hi, qT_lo) for dc in range(DC)
                 for wg in ((Wg_hi, Wg_lo) if q is qT_hi else (Wg_hi,))]
        for t, (wg, dc, q) in enumerate(terms):
            nc.tensor.matmul(ps_r[:], wg[dc][:], q[dc][:, hs],
                             start=(t == 0), stop=(t == len(terms) - 1))
        nc.scalar.activation(routerT[:, hs], ps_r[:], AF.Copy)

    # ---- A3 (interleaved into B blocks): vh key-major -> vh_aug ----
    def emit_vh_kc(kc):
        # shares the ps_a tag: vh use ends before phase C allocates ps_a
        ps = ps_att.tile([P, DK], FP, tag="ps_a", name="ps_vh")
        for dc in range(DC):
            nc.tensor.matmul(ps[:], vTt[:, kc * D + dc * P:kc * D + (dc + 1) * P],
                             Wv[dc][:], start=(dc == 0), stop=(dc == DC - 1))
        nc.scalar.activation(vh_aug[:, kc * VW:kc * VW + DK], ps[:], AF.Copy)

    # ---- B block for one qt: q_all, top-2 select, gather, gates ----
    # Split in two: the PE/ACT/DVE producer part, and a deferred finish
    # (the qsel2/comb8 transposes) emitted one interleave slot later so
    # the PE never waits on the DVE gather.
    b_state = {}

    def emit_B_mm(qt):
        qs = slice(qt * P, (qt + 1) * P)
        # PE: all-expert query projection + bq bias row
        ps_qa = ps_m.tile([P, D], FP, tag="ps")
        for dc in range(DC):
            nc.tensor.matmul(ps_qa[:], qT_hi[dc][:, qs], Wq_f[dc][:],
                             start=(dc == 0), stop=False)
        nc.tensor.matmul(ps_qa[:], ones_row, bq_row, start=False, stop=True)
        qa_b = work.tile([P, D], BF, tag="qa_b")
        nc.scalar.activation(qa_b[:], ps_qa[:], AF.Copy)
        # PE: router logits transpose for this qt
        ps_lg = ps_m.tile([P, E], FP, tag="ps")
        nc.tensor.matmul(ps_lg[:], routerT[:, qs], ident[:E, :E],
                         is_transpose=True)
        lg8 = work.tile([P, E], FP, tag="lg8")
        nc.vector.tensor_copy(lg8[:], ps_lg[:])
        # top-2 selection
        m8 = work.tile([P, E], FP, tag="m8")
        nc.vector.max(out=m8[:], in_=lg8[:])
        i8 = work.tile([P, E], U32, tag="i8")
        nc.vector.max_index(i8[:], m8[:], lg8[:])
        if8 = work.tile([P, 2], FP, tag="if8")
        nc.vector.tensor_copy(if8[:], i8[:, 0:2])
        # softmax pieces: e8 + row sum, top-2 exps
        e8 = work.tile([P, E], FP, tag="e8")
        ssum = work.tile([P, 1], FP, tag="ssum")
        if ACT_ACCUM:
            nc.scalar.activation(e8[:], lg8[:], AF.Exp, accum_out=ssum[:])
        else:
            nc.scalar.activation(e8[:], lg8[:], AF.Exp)
            nc.vector.reduce_sum(ssum[:], e8[:], axis=AX.X)
        gtop = work.tile([P, 2], FP, tag="gtop")
        nc.scalar.activation(gtop[:], m8[:, 0:2], AF.Exp)
        srec = work.tile([P, 1], FP, tag="srec")
        nc.vector.reciprocal(srec[:], ssum[:])
        g = g_all[:, 2 * qt:2 * qt + 2]
        nc.vector.tensor_scalar(g, gtop[:], srec[:], None, op0=OP.mult)
        # masked gather: one fused (iota==e_h)*qa per head, then e-reduce
        qsel2 = work.tile([P, P], BF, tag="qsel2")
        for h in range(2):
            u = work.tile([P, D], BF, tag=f"u{h}", name=f"u{h}")
            nc.vector.scalar_tensor_tensor(
                u[:], iota_e[:], if8[:, h:h + 1], qa_b[:],
                op0=OP.is_equal, op1=OP.mult)
            with nc.allow_low_precision(reason="one-hot masked sum: only one "
                                        "of the 8 summands is nonzero"):
                nc.vector.reduce_sum(qsel2[:, h * DK:(h + 1) * DK],
                                     u[:].rearrange("p (d e) -> p d e", e=E),
                                     axis=AX.X)
        # scatter indices for phase D (int16, (d e) order: 8*d + e_h)
        if USE_GPSIMD_SCATTER:
            for h in range(2):
                nc.vector.tensor_scalar(
                    idx16[:, qt * P + h * DK:qt * P + (h + 1) * DK],
                    iota64x8[:], if8[:, h:h + 1], None, op0=OP.add)
        # gate-weighted one-hot combine row (for bo' bias matmul)
        comb8 = work.tile([P, E], BF, tag="comb8")
        tmp8 = work.tile([P, E], BF, tag="tmp8")
        nc.vector.scalar_tensor_tensor(
            comb8[:], iota8[:], if8[:, 0:1],
            g[:, 0:1].to_broadcast((P, E)), op0=OP.is_equal, op1=OP.mult)
        nc.vector.scalar_tensor_tensor(
            tmp8[:], iota8[:], if8[:, 1:2],
            g[:, 1:2].to_broadcast((P, E)), op0=OP.is_equal, op1=OP.mult)
        nc.vector.tensor_tensor(comb8[:], comb8[:], tmp8[:], op=OP.add)
        b_state[qt] = (qsel2, comb8)

    def emit_B_fin(qt):
        qs = slice(qt * P, (qt + 1) * P)
        qsel2, comb8 = b_state.pop(qt)
        ps_qsT = ps_m.tile([P, P], BF, tag="ps")
        nc.tensor.matmul(ps_qsT[:], qsel2[:], ident_b[:], is_transpose=True)
        nc.vector.tensor_copy(qselT2[:, qs], ps_qsT[:])
        ps_cbT = ps_m.tile([E, P], BF, tag="ps")
        nc.tensor.matmul(ps_cbT[:], comb8[:], ident_b[:], is_transpose=True)
        nc.vector.tensor_copy(combT[:, qs], ps_cbT[:])

    def emit_B(qt):
        emit_B_mm(qt)
        emit_B_fin(qt)

    # ---- D1 for one qt: transpose attn back, scale, scatter to cm ----
    cms = {}
    def emit_D1(qt):
        qs = slice(qt * P, (qt + 1) * P)
        h_at = []
        for h in range(2):
            ps_at = ps_m.tile([P, VW], BF, tag="ps")
            nc.tensor.matmul(ps_at[:], attnT[h][:, qs], ident_b[:VW, :VW],
                             is_transpose=True)
            at = dpool.tile([P, VW], FP, tag=f"at{qt % 2}_{h}",
                            name=f"at{qt % 2}_{h}")
            nc.vector.tensor_copy(at[:], ps_at[:])
            h_at.append(at)
        ds = dpool.tile([P, P], BF, tag=f"ds{qt % 2}", name=f"ds{qt % 2}")
        for h in range(2):
            # s = gate / denom, then ds = at * s (keep gpsimd scatter-only:
            # mixing gpsimd libraries forces ~2us IRAM reloads per call)
            dinv = work.tile([P, 1], FP, tag="dinv")
            nc.vector.reciprocal(dinv[:], h_at[h][:, DK:DK + 1])
            s = work.tile([P, 1], FP, tag="s")
            nc.vector.tensor_tensor(s[:], g_all[:, 2 * qt + h:2 * qt + h + 1],
                                    dinv[:], op=OP.mult)
            nc.vector.tensor_scalar(ds[:, h * DK:(h + 1) * DK],
                                    h_at[h][:, 0:DK], s[:], None, op0=OP.mult)
        cm = dpool.tile([P, D], BF, tag=f"cm{qt % 2}", name=f"cm{qt % 2}")
        nc.gpsimd.local_scatter(cm[:], ds[:], idx16[:, qt * P:(qt + 1) * P],
                                channels=P, num_elems=D, num_idxs=P)
        cms[qt] = cm

    # ---- D3 for one qt: transpose cm, output projection, store ----
    def emit_D3(qt, late):
        qs = slice(qt * P, (qt + 1) * P)
        cm = cms.pop(qt)
        cTs = []
        tp = ps_log if late else ps_m
        for ci in range(DC):
            ps_ct = tp.tile([P, P], BF, tag="ps" if tp is ps_m else "ps_lg2",
                            name="ps_ct")
            nc.tensor.matmul(ps_ct[:], cm[:, ci * P:(ci + 1) * P], ident_b[:],
                             is_transpose=True)
            cT = work.tile([P, P], BF, tag=f"cT{ci}", name=f"cT{ci}")
            if ci % 2 == 0:
                nc.vector.tensor_copy(cT[:], ps_ct[:])
            else:
                nc.scalar.activation(cT[:], ps_ct[:], AF.Copy)
            cTs.append(cT)
        ps_o = ps_m.tile([P, D], FP, tag="ps")
        for ci in range(DC):
            nc.tensor.matmul(ps_o[:], cTs[ci][:], Wo_f[ci][:],
                             start=(ci == 0), stop=False)
        nc.tensor.matmul(ps_o[:], combT[:, qs], bo8, start=False, stop=True)
        o = work.tile([P, D], BF, tag="o")
        nc.scalar.activation(o[:], ps_o[:], AF.Copy)
        nc.sync.dma_start(out_d[qs, :], o[:])

    # ---- phase C for one half, with interleaved extra blocks ----
    def emit_C(half, extras):
        hs = slice(half * HD, (half + 1) * HD)
        ps_a = [ps_att.tile([VW, HD], FP, tag="ps_a", name=f"ps_a{h}")
                for h in range(2)]
        pending = None
        for kc in range(NKC):
            ps_lg2 = ps_log.tile([P, 2 * HD], FP, tag="ps_lg2")
            for h in range(2):
                rg = slice(h * DK, (h + 1) * DK)
                nc.tensor.matmul(ps_lg2[:, h * HD:(h + 1) * HD],
                                 khT2[rg, kc * P:(kc + 1) * P],
                                 qselT2[rg, hs], start=True, stop=True)
            if pending is not None:
                pkc, pex = pending
                for h in range(2):
                    nc.tensor.matmul(ps_a[h][:],
                                     vh_aug[:, pkc * VW:(pkc + 1) * VW],
                                     pex[:, h * HD:(h + 1) * HD],
                                     start=(pkc == 0), stop=(pkc == NKC - 1),
                                     skip_group_check=True)
            ex = expp.tile([P, 2 * HD], BF, tag="ex")
            nc.scalar.activation(ex[:], ps_lg2[:], AF.Exp)
            pending = (kc, ex)
            if kc in extras:
                extras[kc]()
        pkc, pex = pending
        for h in range(2):
            nc.tensor.matmul(ps_a[h][:], vh_aug[:, pkc * VW:(pkc + 1) * VW],
                             pex[:, h * HD:(h + 1) * HD],
                             start=(pkc == 0), stop=(pkc == NKC - 1),
                             skip_group_check=True)
        for h in range(2):
            nc.vector.tensor_copy(attnT[h][:, hs], ps_a[h][:])

    # ================= emission =================
    # B blocks for qts 0-3 (transposes deferred one block), vh interleaved
    for qt in range(4):
        emit_B_mm(qt)
        if qt > 0:
            emit_B_fin(qt - 1)
        for kc in range(4 * qt, 4 * qt + 4):
            emit_vh_kc(kc)
    emit_B_fin(3)

    # C half 0, with B(4..7) interleaved into the matmul stream
    emit_C(0, {1: lambda: emit_B_mm(4), 3: lambda: emit_B_fin(4),
               5: lambda: emit_B_mm(5), 7: lambda: emit_B_fin(5),
               9: lambda: emit_B_mm(6), 11: lambda: emit_B_fin(6),
               13: lambda: emit_B_mm(7), 15: lambda: emit_B_fin(7)})

    # C half 1, with D(0..2) interleaved (their attnT half-0 data is ready)
    emit_C(1, {1: lambda: emit_D1(0), 3: lambda: emit_D1(1),
               5: lambda: emit_D3(0, False), 7: lambda: emit_D1(2),
               9: lambda: emit_D3(1, False), 11: lambda: emit_D1(3),
               13: lambda: emit_D3(2, False)})

    # drain: remaining D, staggered so scatters run ahead of D3 matmuls
    emit_D1(4)
    emit_D1(5)
    emit_D3(3, True)
    emit_D1(6)
    emit_D3(4, True)
    emit_D1(7)
    emit_D3(5, True)
    emit_D3(6, True)
    emit_D3(7, True)


_PROGRAM = None


def get_program():
    global _PROGRAM
    if _PROGRAM is None:
        nc = bacc.Bacc("TRN2", target_bir_lowering=False, debug=False,
                       enable_asserts=False, num_devices=8)
        from contextlib import ExitStack
        with tile.TileContext(nc) as tc, ExitStack() as ctx:
            _emit(nc, tc, ctx)
        nc.compile()
        _PROGRAM = nc
    return _PROGRAM


def make_in_maps(query, key, value, Wg, Wk, bk, Wv, bv, Wq, bq, Wo, bo):
    import ml_dtypes
    BFNP = ml_dtypes.bfloat16

    def hilo(x):
        x = np.asarray(x, np.float32)
        hi = x.astype(BFNP)
        lo = (x - hi.astype(np.float32)).astype(BFNP)
        return hi, lo

    Wg_hi, Wg_lo = hilo(Wg)
    # kh pre-scaled by 1/8 == 1/sqrt(DK); doubled for the two head row-groups
    Wk2s = np.concatenate([np.asarray(Wk), np.asarray(Wk)], axis=1) * 0.125
    # bk shifts all logits of a query equally -> softmax-invariant: dropped.
    # (d e) ordering (col/row index = d*E + e)
    Wq_f = np.asarray(Wq).transpose(1, 2, 0).reshape(D, DK * E)
    Wo_f = np.asarray(Wo).transpose(1, 0, 2).reshape(DK * E, D)
    bq_f = np.asarray(bq).T.reshape(DK * E)
    # bv folds into bo since attention weights sum to 1
    bo_p = np.asarray(bo) + np.einsum('d,edm->em', np.asarray(bv, np.float32),
                                      np.asarray(Wo, np.float32))

    w0a = np.zeros((P, _W0A_COLS), BFNP)
    def put(name, rows, arr):
        base = _W0A[name]
        arr = np.asarray(arr, BFNP)
        w0a[rows, base:base + arr.shape[-1]] = arr
    for dc in range(DC):
        rows = slice(0, P)
        w0a[:, _W0A["Wk2s"] + dc * P:_W0A["Wk2s"] + (dc + 1) * P] = \
            np.asarray(Wk2s[dc * P:(dc + 1) * P, :], BFNP)
        w0a[:, _W0A["Wv"] + dc * DK:_W0A["Wv"] + (dc + 1) * DK] = \
            np.asarray(Wv, np.float32)[dc * P:(dc + 1) * P, :].astype(BFNP)
        w0a[:, _W0A["Wg_hi"] + dc * E:_W0A["Wg_hi"] + (dc + 1) * E] = \
            Wg_hi[dc * P:(dc + 1) * P, :]
        w0a[:, _W0A["Wg_lo"] + dc * E:_W0A["Wg_lo"] + (dc + 1) * E] = \
            Wg_lo[dc * P:(dc + 1) * P, :]
    w0a[:, _W0A["ones"]:_W0A["ones"] + P] = np.ones((P, P), BFNP)
    put("bq_row", 0, bq_f)
    put("bo", slice(0, E), bo_p)

    w0b = np.zeros((P, _W0B_COLS), BFNP)
    for dc in range(DC):
        w0b[:, dc * D:(dc + 1) * D] = \
            np.asarray(Wq_f, np.float32)[dc * P:(dc + 1) * P, :].astype(BFNP)
        w0b[:, DC * D + dc * D:DC * D + (dc + 1) * D] = \
            np.asarray(Wo_f, np.float32)[dc * P:(dc + 1) * P, :].astype(BFNP)

    def pack_chunks(x, n):  # [512, N] -> [128, 4*N] (dc-major columns)
        out = np.empty((P, DC * n), x.dtype)
        for dc in range(DC):
            out[:, dc * n:(dc + 1) * n] = x[dc * P:(dc + 1) * P, :]
        return np.ascontiguousarray(out)

    def pack_blocks(x, nblk, blkcols):  # [512, N] -> [128, nblk*(4*blkcols)]
        # block j = [dc0 cols | dc1 cols | dc2 cols | dc3 cols]
        out = np.empty((P, DC * nblk * blkcols), x.dtype)
        for j in range(nblk):
            for dc in range(DC):
                off = j * DC * blkcols + dc * blkcols
                out[:, off:off + blkcols] = \
                    x[dc * P:(dc + 1) * P, j * blkcols:(j + 1) * blkcols]
        return np.ascontiguousarray(out)

    shared = {"w0a": np.ascontiguousarray(w0a), "w0b": np.ascontiguousarray(w0b)}
    in_maps = []
    for b in range(4):
        kT = np.asarray(key[b], np.float32).T.astype(BFNP)     # [512, 2048]
        vT = np.asarray(value[b], np.float32).T.astype(BFNP)
        kTn = pack_blocks(kT, 4, HD)      # ncH-major blocks of 512 keys
        vTn = pack_blocks(vT, NKC, P)     # kc-major blocks of 128 keys
        for h in range(2):
            qhi, qlo = hilo(np.asarray(query[b][h * NQ:(h + 1) * NQ, :]).T)
            in_maps.append({"kTn": kTn, "vTn": vTn,
                            "qhi": pack_chunks(qhi, NQ),
                            "qlo": pack_chunks(qlo, NQ), **shared})
    return in_maps


def kernel(query, key, value, Wg, Wk, bk, Wv, bv, Wq, bq, Wo, bo):
    in_maps = make_in_maps(query, key, value, Wg, Wk, bk, Wv, bv, Wq, bq, Wo, bo)
    nc = get_program()
    res = bass_utils.run_bass_kernel_spmd(nc, in_maps, core_ids=list(range(8)))
    outs = [np.asarray(res.results[c]["out"], np.float32) for c in range(8)]
    return np.concatenate(outs, axis=0).reshape(4, T, D)


# revision 22
# speedup vs baseline: 1.5739x; 1.0129x over previous
"""MoE-routed multi-head attention kernel for 8 Trainium2 NeuronCores.

Problem shape (hardcoded):
  query/key/value: [4, 2048, 512] f32
  Wg [512,8], Wk/Wv [512,64], Wq [8,512,64], Wo [8,64,512], biases.
  TOP_K=2 routed experts act as the two attention heads.

Sharding: core c = 2*b + h handles batch b, query-half h (1024 query tokens),
with the full 2048 keys/values of batch b.

Key structural choices (v2):
  - bk is dropped: adding bk to kh shifts every logit of a query by a
    constant, which softmax ignores.  bv is folded into bo on the host
    (bo' = bo + bv @ Wo_e) since attn rows sum to 1 before the gate.
  - vh is computed directly key-major (vT chunks stationary, Wv moving),
    so no PE transposes of vh are needed.
  - Router logits (3-term bf16 hi/lo, fp32-exact for top-2 selection) are
    computed T-layout with tiny Wg-stationary loads, transposed per qt.
  - The expert gather (qsel from q_all) is a chain of 8 fused
    scalar_tensor_tensor ops per head; the expert scatter (attn -> cm)
    is a gpsimd local_scatter with per-token int16 indices.
  - Phase C: per kc one [128,1024] fp32 2-bank PSUM logits tile (both
    heads via row-grouped concurrent matmuls), one [128,1024] exp, and
    2 attention matmuls, software-pipelined.  B-blocks for qts 4-7 are
    interleaved into C(half0)'s PE stream, D-blocks into C(half1)'s, so
    no engine idles and the PE never sees a matmul-free window (HAM).
  - Output is written bf16; host casts to fp32.
"""

import numpy as np

import concourse.bass as bass
import concourse.mybir as mybir
import concourse.tile as tile
from concourse import bacc
from concourse import bass_utils
from concourse.masks import make_identity

P = 128
D = 512          # d_model
T = 2048         # kv tokens per core (full batch)
NQ = 1024        # query tokens per core
E = 8            # experts
DK = 64          # head dim
DC = D // P      # 4 contraction chunks
NKC = T // P     # 16 key chunks
NQT = NQ // P    # 8 query tiles
VW = DK + 1      # vh columns + ones column (denominator trick)
HD = 512         # phase-C column granularity (half of NQ)

FP = mybir.dt.float32
U32 = mybir.dt.uint32
I16 = mybir.dt.int16
BF = mybir.dt.bfloat16
AF = mybir.ActivationFunctionType
OP = mybir.AluOpType
AX = mybir.AxisListType

USE_GPSIMD_SCATTER = True
ACT_ACCUM = True         # activation accum_out for softmax row sums

# ---- w0a packed layout (bf16 columns) ----
_W0A = {}
_off = 0
for name, cols in [("Wk2s", DC * P), ("Wv", DC * DK), ("Wg_hi", DC * E),
                   ("Wg_lo", DC * E), ("ones", P), ("bq_row", D), ("bo", D)]:
    _W0A[name] = _off
    _off += cols
_W0A_COLS = _off
_W0B_COLS = 2 * DC * D   # Wq_f (e d) | Wo_f (e d)


def _emit(nc, tc, ctx):
    const = ctx.enter_context(tc.tile_pool(name="const", bufs=1))
    persist = ctx.enter_context(tc.tile_pool(name="persist", bufs=1))
    work = ctx.enter_context(tc.tile_pool(name="work", bufs=3))
    expp = ctx.enter_context(tc.tile_pool(name="expp", bufs=3))
    dpool = ctx.enter_context(tc.tile_pool(name="dpool", bufs=2))
    ps_log = ctx.enter_context(tc.tile_pool(name="ps_log", bufs=2, space="PSUM"))
    ps_att = ctx.enter_context(tc.tile_pool(name="ps_att", bufs=2, space="PSUM"))
    ps_m = ctx.enter_context(tc.tile_pool(name="ps_m", bufs=2, space="PSUM"))

    dram = {}
    for name, shape, dt in [
        ("w0a", [P, _W0A_COLS], BF), ("w0b", [P, _W0B_COLS], BF),
        ("kTn", [P, DC * T], BF), ("vTn", [P, DC * T], BF),
        ("qhi", [P, DC * NQ], BF), ("qlo", [P, DC * NQ], BF),
    ]:
        dram[name] = nc.dram_tensor(name, shape, dt, kind="ExternalInput").ap()
    out_d = nc.dram_tensor("out", [NQ, D], BF, kind="ExternalOutput").ap()

    # ---- DMAs, ordered by first use; kT split so khT2 starts early ----
    w0a = const.tile([P, _W0A_COLS], BF, tag="w0a")
    nc.sync.dma_start(w0a[:, :DC * P], dram["w0a"][:, :DC * P])  # Wk2s first
    nc.sync.dma_start(w0a[:, DC * P:], dram["w0a"][:, DC * P:])
    kTt = persist.tile([P, DC * T], BF, tag="kTt")
    for j in range(4):
        cs = slice(j * T, (j + 1) * T)
        nc.sync.dma_start(kTt[:, cs], dram["kTn"][:, cs])
    qhi_t = persist.tile([P, DC * NQ], BF, tag="qhi")
    nc.sync.dma_start(qhi_t[:], dram["qhi"])
    qlo_t = persist.tile([P, DC * NQ], BF, tag="qlo")
    nc.sync.dma_start(qlo_t[:], dram["qlo"])
    w0b = const.tile([P, _W0B_COLS], BF, tag="w0b")
    nc.sync.dma_start(w0b[:], dram["w0b"])
    vTt = persist.tile([P, DC * T], BF, tag="vTt")
    for j in range(4):
        cs = slice(j * T, (j + 1) * T)
        nc.sync.dma_start(vTt[:, cs], dram["vTn"][:, cs])

    def w0(name, r0, r1, c0, c1):
        base = _W0A[name]
        return w0a[r0:r1, base + c0:base + c1]

    Wk2s = {dc: w0("Wk2s", 0, P, dc * P, (dc + 1) * P) for dc in range(DC)}
    Wv = {dc: w0("Wv", 0, P, dc * DK, (dc + 1) * DK) for dc in range(DC)}
    Wg_hi = {dc: w0("Wg_hi", 0, P, dc * E, (dc + 1) * E) for dc in range(DC)}
    Wg_lo = {dc: w0("Wg_lo", 0, P, dc * E, (dc + 1) * E) for dc in range(DC)}
    ones_blk = w0("ones", 0, P, 0, P)
    ones_row = w0("ones", 0, 1, 0, P)        # [1,128] lhsT for bias matmul
    bq_row = w0("bq_row", 0, 1, 0, D)        # [1,512] (e d) order
    bo8 = w0("bo", 0, E, 0, D)               # [8,512] bo' = bo + bv@Wo
    Wq_f = {dc: w0b[:, dc * D:(dc + 1) * D] for dc in range(DC)}
    Wo_f = {dc: w0b[:, DC * D + dc * D:DC * D + (dc + 1) * D] for dc in range(DC)}
    qT_hi = {dc: qhi_t[:, dc * NQ:(dc + 1) * NQ] for dc in range(DC)}
    qT_lo = {dc: qlo_t[:, dc * NQ:(dc + 1) * NQ] for dc in range(DC)}

    # ---- constants ----
    ident = const.tile([P, P], FP, tag="ident")
    make_identity(nc, ident[:])
    ident_b = const.tile([P, P], BF, tag="ident_b")
    make_identity(nc, ident_b[:])
    iota8 = const.tile([P, E], FP, tag="iota8")
    nc.gpsimd.iota(iota8[:], pattern=[[1, E]], channel_multiplier=0,
                   allow_small_or_imprecise_dtypes=True)
    iota64x8 = const.tile([P, DK], FP, tag="iota64x8")   # 0,8,16,...,504
    nc.gpsimd.iota(iota64x8[:], pattern=[[8, DK]], channel_multiplier=0,
                   allow_small_or_imprecise_dtypes=True)
    iota_e = const.tile([P, DK * E], BF, tag="iota_e")   # expert id, (d e) cols
    nc.gpsimd.iota(iota_e[:].rearrange("p (d e) -> p d e", e=E),
                   pattern=[[0, DK], [1, E]], channel_multiplier=0,
                   allow_small_or_imprecise_dtypes=True)

    # ---- persistent intermediates ----
    khT2 = persist.tile([P, T], BF, tag="khT2")
    vh_aug = persist.tile([P, NKC * VW], BF, tag="vh_aug")
    routerT = persist.tile([E, NQ], FP, tag="routerT")
    qselT2 = persist.tile([P, NQ], BF, tag="qselT2")
    combT = persist.tile([E, NQ], BF, tag="combT")
    attnT = [persist.tile([VW, NQ], BF, tag=f"attnT{h}", name=f"attnT{h}")
             for h in range(2)]
    g_all = persist.tile([P, 2 * NQT], FP, tag="g_all")    # gates, 2 per qt
    idx16 = persist.tile([P, P * NQT], I16, tag="idx16")   # scatter idxs per qt

    # vh_aug ones columns (col 64 of each kc block)
    nc.vector.tensor_copy(
        vh_aug[:].rearrange("p (c w) -> p c w", w=VW)[:, :, DK],
        ones_blk[:, 0:NKC])

    # ---- A1: khT2 [128, T] (head-doubled kh, pre-scaled by 1/8) ----
    def emit_khT2_chunk(j):
        cs = slice(j * HD, (j + 1) * HD)
        ps = ps_m.tile([P, HD], FP, tag="ps")
        for dc in range(DC):
            nc.tensor.matmul(ps[:], Wk2s[dc][:], kTt[:, j * T + dc * HD:
                                                     j * T + (dc + 1) * HD],
                             start=(dc == 0), stop=(dc == DC - 1))
        nc.scalar.activation(khT2[:, cs], ps[:], AF.Copy)

    for j in range(4):
        emit_khT2_chunk(j)

    # ---- A2: router logits, T-layout, 3-term hi/lo (fp32-exact) ----
    for half in range(2):
        hs = slice(half * HD, (half + 1) * HD)
        ps_r = ps_m.tile([E, HD], FP, tag="ps")
        terms = [(wg, dc, q) for q in (qT_hi, qT_lo) for dc in range(DC)
                 for wg in ((Wg_hi, Wg_lo) if q is qT_hi else (Wg_hi,))]
        for t, (wg, dc, q) in enumerate(terms):
            nc.tensor.matmul(ps_r[:], wg[dc][:], q[dc][:, hs],
                             start=(t == 0), stop=(t == len(terms) - 1))
        nc.scalar.activation(routerT[:, hs], ps_r[:], AF.Copy)

    # ---- A3 (interleaved into B blocks): vh key-major -> vh_aug ----
    def emit_vh_kc(kc):
        # shares the ps_a tag: vh use ends before phase C allocates ps_a
        ps = ps_att.tile([P, DK], FP, tag="ps_a", name="ps_vh")
        for dc in range(DC):
            nc.tensor.matmul(ps[:], vTt[:, kc * D + dc * P:kc * D + (dc + 1) * P],
                             Wv[dc][:], start=(dc == 0), stop=(dc == DC - 1))
        nc.scalar.activation(vh_aug[:, kc * VW:kc * VW + DK], ps[:], AF.Copy)

    # ---- B block for one qt: q_all, top-2 select, gather, gates ----
    # Split in two: the PE/ACT/DVE producer part, and a deferred finish
    # (the qsel2/comb8 transposes) emitted one interleave slot later so
    # the PE never waits on the DVE gather.
    b_state = {}

    def emit_B_mm(qt):
        qs = slice(qt * P, (qt + 1) * P)
        # PE: all-expert query projection + bq bias row
        ps_qa = ps_m.tile([P, D], FP, tag="ps")
        for dc in range(DC):
            nc.tensor.matmul(ps_qa[:], qT_hi[dc][:, qs], Wq_f[dc][:],
                             start=(dc == 0), stop=False)
        nc.tensor.matmul(ps_qa[:], ones_row, bq_row, start=False, stop=True)
        qa_b = work.tile([P, D], BF, tag="qa_b")
        if qt < 4:   # qts 4-7 run inside C half0 where ACT is exp-bound
            nc.scalar.activation(qa_b[:], ps_qa[:], AF.Copy)
        else:
            nc.vector.tensor_copy(qa_b[:], ps_qa[:])
        # PE: router logits transpose for this qt
        ps_lg = ps_m.tile([P, E], FP, tag="ps")
        nc.tensor.matmul(ps_lg[:], routerT[:, qs], ident[:E, :E],
                         is_transpose=True)
        lg8 = work.tile([P, E], FP, tag="lg8")
        nc.vector.tensor_copy(lg8[:], ps_lg[:])
        # top-2 selection
        m8 = work.tile([P, E], FP, tag="m8")
        nc.vector.max(out=m8[:], in_=lg8[:])
        i8 = work.tile([P, E], U32, tag="i8")
        nc.vector.max_index(i8[:], m8[:], lg8[:])
        if8 = work.tile([P, 2], FP, tag="if8")
        nc.vector.tensor_copy(if8[:], i8[:, 0:2])
        # softmax pieces: e8 + row sum, top-2 exps
        e8 = work.tile([P, E], FP, tag="e8")
        ssum = work.tile([P, 1], FP, tag="ssum")
        if ACT_ACCUM:
            nc.scalar.activation(e8[:], lg8[:], AF.Exp, accum_out=ssum[:])
        else:
            nc.scalar.activation(e8[:], lg8[:], AF.Exp)
            nc.vector.reduce_sum(ssum[:], e8[:], axis=AX.X)
        gtop = work.tile([P, 2], FP, tag="gtop")
        nc.scalar.activation(gtop[:], m8[:, 0:2], AF.Exp)
        srec = work.tile([P, 1], FP, tag="srec")
        nc.vector.reciprocal(srec[:], ssum[:])
        g = g_all[:, 2 * qt:2 * qt + 2]
        nc.vector.tensor_scalar(g, gtop[:], srec[:], None, op0=OP.mult)
        # masked gather: one fused (iota==e_h)*qa per head, then e-reduce
        qsel2 = work.tile([P, P], BF, tag="qsel2")
        for h in range(2):
            u = work.tile([P, D], BF, tag=f"u{h}", name=f"u{h}")
            nc.vector.scalar_tensor_tensor(
                u[:], iota_e[:], if8[:, h:h + 1], qa_b[:],
                op0=OP.is_equal, op1=OP.mult)
            with nc.allow_low_precision(reason="one-hot masked sum: only one "
                                        "of the 8 summands is nonzero"):
                nc.vector.reduce_sum(qsel2[:, h * DK:(h + 1) * DK],
                                     u[:].rearrange("p (d e) -> p d e", e=E),
                                     axis=AX.X)
        # scatter indices for phase D (int16, (d e) order: 8*d + e_h)
        if USE_GPSIMD_SCATTER:
            for h in range(2):
                nc.vector.tensor_scalar(
                    idx16[:, qt * P + h * DK:qt * P + (h + 1) * DK],
                    iota64x8[:], if8[:, h:h + 1], None, op0=OP.add)
        # gate-weighted one-hot combine row (for bo' bias matmul)
        comb8 = work.tile([P, E], BF, tag="comb8")
        tmp8 = work.tile([P, E], BF, tag="tmp8")
        nc.vector.scalar_tensor_tensor(
            comb8[:], iota8[:], if8[:, 0:1],
            g[:, 0:1].to_broadcast((P, E)), op0=OP.is_equal, op1=OP.mult)
        nc.vector.scalar_tensor_tensor(
            tmp8[:], iota8[:], if8[:, 1:2],
            g[:, 1:2].to_broadcast((P, E)), op0=OP.is_equal, op1=OP.mult)
        nc.vector.tensor_tensor(comb8[:], comb8[:], tmp8[:], op=OP.add)
        b_state[qt] = (qsel2, comb8)

    def emit_B_fin(qt):
        qs = slice(qt * P, (qt + 1) * P)
        qsel2, comb8 = b_state.pop(qt)
        ps_qsT = ps_m.tile([P, P], BF, tag="ps")
        nc.tensor.matmul(ps_qsT[:], qsel2[:], ident_b[:], is_transpose=True)
        nc.vector.tensor_copy(qselT2[:, qs], ps_qsT[:])
        ps_cbT = ps_m.tile([E, P], BF, tag="ps")
        nc.tensor.matmul(ps_cbT[:], comb8[:], ident_b[:], is_transpose=True)
        nc.vector.tensor_copy(combT[:, qs], ps_cbT[:])

    def emit_B(qt):
        emit_B_mm(qt)
        emit_B_fin(qt)

    # ---- D1 for one qt: transpose attn back, scale, scatter to cm ----
    cms = {}
    def emit_D1(qt):
        qs = slice(qt * P, (qt + 1) * P)
        h_at = []
        for h in range(2):
            ps_at = ps_m.tile([P, VW], BF, tag="ps")
            nc.tensor.matmul(ps_at[:], attnT[h][:, qs], ident_b[:VW, :VW],
                             is_transpose=True)
            at = dpool.tile([P, VW], FP, tag=f"at{qt % 2}_{h}",
                            name=f"at{qt % 2}_{h}")
            nc.vector.tensor_copy(at[:], ps_at[:])
            h_at.append(at)
        ds = dpool.tile([P, P], BF, tag=f"ds{qt % 2}", name=f"ds{qt % 2}")
        for h in range(2):
            # s = gate / denom, then ds = at * s (keep gpsimd scatter-only:
            # mixing gpsimd libraries forces ~2us IRAM reloads per call)
            dinv = work.tile([P, 1], FP, tag="dinv")
            nc.vector.reciprocal(dinv[:], h_at[h][:, DK:DK + 1])
            s = work.tile([P, 1], FP, tag="s")
            nc.vector.tensor_tensor(s[:], g_all[:, 2 * qt + h:2 * qt + h + 1],
                                    dinv[:], op=OP.mult)
            nc.vector.tensor_scalar(ds[:, h * DK:(h + 1) * DK],
                                    h_at[h][:, 0:DK], s[:], None, op0=OP.mult)
        cm = dpool.tile([P, D], BF, tag=f"cm{qt % 2}", name=f"cm{qt % 2}")
        nc.gpsimd.local_scatter(cm[:], ds[:], idx16[:, qt * P:(qt + 1) * P],
                                channels=P, num_elems=D, num_idxs=P)
        cms[qt] = cm

    # ---- D3 for one qt: transpose cm, output projection, store ----
    def emit_D3(qt, late):
        qs = slice(qt * P, (qt + 1) * P)
        cm = cms.pop(qt)
        cTs = []
        tp = ps_log if late else ps_m
        for ci in range(DC):
            ps_ct = tp.tile([P, P], BF, tag="ps" if tp is ps_m else "ps_lg2",
                            name="ps_ct")
            nc.tensor.matmul(ps_ct[:], cm[:, ci * P:(ci + 1) * P], ident_b[:],
                             is_transpose=True)
            cT = work.tile([P, P], BF, tag=f"cT{ci}", name=f"cT{ci}")
            if ci % 2 == 0:
                nc.vector.tensor_copy(cT[:], ps_ct[:])
            else:
                nc.scalar.activation(cT[:], ps_ct[:], AF.Copy)
            cTs.append(cT)
        ps_o = ps_m.tile([P, D], FP, tag="ps")
        for ci in range(DC):
            nc.tensor.matmul(ps_o[:], cTs[ci][:], Wo_f[ci][:],
                             start=(ci == 0), stop=False)
        nc.tensor.matmul(ps_o[:], combT[:, qs], bo8, start=False, stop=True)
        o = work.tile([P, D], BF, tag="o")
        nc.scalar.activation(o[:], ps_o[:], AF.Copy)
        nc.sync.dma_start(out_d[qs, :], o[:])

    # ---- phase C for one half, with interleaved extra blocks ----
    def emit_C(half, extras):
        hs = slice(half * HD, (half + 1) * HD)
        ps_a = [ps_att.tile([VW, HD], FP, tag="ps_a", name=f"ps_a{h}")
                for h in range(2)]
        pending = None
        for kc in range(NKC):
            ps_lg2 = ps_log.tile([P, 2 * HD], FP, tag="ps_lg2")
            for h in range(2):
                rg = slice(h * DK, (h + 1) * DK)
                nc.tensor.matmul(ps_lg2[:, h * HD:(h + 1) * HD],
                                 khT2[rg, kc * P:(kc + 1) * P],
                                 qselT2[rg, hs], start=True, stop=True)
            if pending is not None:
                pkc, pex = pending
                for h in range(2):
                    nc.tensor.matmul(ps_a[h][:],
                                     vh_aug[:, pkc * VW:(pkc + 1) * VW],
                                     pex[:, h * HD:(h + 1) * HD],
                                     start=(pkc == 0), stop=(pkc == NKC - 1),
                                     skip_group_check=True)
            ex = expp.tile([P, 2 * HD], BF, tag="ex")
            nc.scalar.activation(ex[:], ps_lg2[:], AF.Exp)
            pending = (kc, ex)
            if kc in extras:
                extras[kc]()
        pkc, pex = pending
        for h in range(2):
            nc.tensor.matmul(ps_a[h][:], vh_aug[:, pkc * VW:(pkc + 1) * VW],
                             pex[:, h * HD:(h + 1) * HD],
                             start=(pkc == 0), stop=(pkc == NKC - 1),
                             skip_group_check=True)
        for h in range(2):
            nc.vector.tensor_copy(attnT[h][:, hs], ps_a[h][:])

    # ================= emission =================
    # B blocks for qts 0-3 (transposes deferred one block), vh interleaved
    for qt in range(4):
        emit_B_mm(qt)
        if qt > 0:
            emit_B_fin(qt - 1)
        for kc in range(4 * qt, 4 * qt + 4):
            emit_vh_kc(kc)
    emit_B_fin(3)

    # C half 0, with B(4..7) interleaved into the matmul stream
    emit_C(0, {1: lambda: emit_B_mm(4), 3: lambda: emit_B_fin(4),
               5: lambda: emit_B_mm(5), 7: lambda: emit_B_fin(5),
               9: lambda: emit_B_mm(6), 11: lambda: emit_B_fin(6),
               13: lambda: emit_B_mm(7), 15: lambda: emit_B_fin(7)})

    # C half 1, with D(0..3) interleaved (their attnT half-0 data is ready)
    emit_C(1, {1: lambda: emit_D1(0), 3: lambda: emit_D1(1),
               5: lambda: emit_D3(0, False), 7: lambda: emit_D1(2),
               9: lambda: emit_D3(1, False), 11: lambda: emit_D1(3),
               13: lambda: emit_D3(2, False), 15: lambda: emit_D3(3, False)})

    # drain: remaining D, staggered so scatters run ahead of D3 matmuls
    emit_D1(4)
    emit_D1(5)
    emit_D3(4, True)
    emit_D1(6)
    emit_D3(5, True)
    emit_D1(7)
    emit_D3(6, True)
    emit_D3(7, True)


_PROGRAM = None


def get_program():
    global _PROGRAM
    if _PROGRAM is None:
        nc = bacc.Bacc("TRN2", target_bir_lowering=False, debug=False,
                       enable_asserts=False, num_devices=8)
        from contextlib import ExitStack
        with tile.TileContext(nc) as tc, ExitStack() as ctx:
            _emit(nc, tc, ctx)
        nc.compile()
        _PROGRAM = nc
    return _PROGRAM


def make_in_maps(query, key, value, Wg, Wk, bk, Wv, bv, Wq, bq, Wo, bo):
    import ml_dtypes
    BFNP = ml_dtypes.bfloat16

    def hilo(x):
        x = np.asarray(x, np.float32)
        hi = x.astype(BFNP)
        lo = (x - hi.astype(np.float32)).astype(BFNP)
        return hi, lo

    Wg_hi, Wg_lo = hilo(Wg)
    # kh pre-scaled by 1/8 == 1/sqrt(DK); doubled for the two head row-groups
    Wk2s = np.concatenate([np.asarray(Wk), np.asarray(Wk)], axis=1) * 0.125
    # bk shifts all logits of a query equally -> softmax-invariant: dropped.
    # (d e) ordering (col/row index = d*E + e)
    Wq_f = np.asarray(Wq).transpose(1, 2, 0).reshape(D, DK * E)
    Wo_f = np.asarray(Wo).transpose(1, 0, 2).reshape(DK * E, D)
    bq_f = np.asarray(bq).T.reshape(DK * E)
    # bv folds into bo since attention weights sum to 1
    bo_p = np.asarray(bo) + np.einsum('d,edm->em', np.asarray(bv, np.float32),
                                      np.asarray(Wo, np.float32))

    w0a = np.zeros((P, _W0A_COLS), BFNP)
    def put(name, rows, arr):
        base = _W0A[name]
        arr = np.asarray(arr, BFNP)
        w0a[rows, base:base + arr.shape[-1]] = arr
    for dc in range(DC):
        rows = slice(0, P)
        w0a[:, _W0A["Wk2s"] + dc * P:_W0A["Wk2s"] + (dc + 1) * P] = \
            np.asarray(Wk2s[dc * P:(dc + 1) * P, :], BFNP)
        w0a[:, _W0A["Wv"] + dc * DK:_W0A["Wv"] + (dc + 1) * DK] = \
            np.asarray(Wv, np.float32)[dc * P:(dc + 1) * P, :].astype(BFNP)
        w0a[:, _W0A["Wg_hi"] + dc * E:_W0A["Wg_hi"] + (dc + 1) * E] = \
            Wg_hi[dc * P:(dc + 1) * P, :]
        w0a[:, _W0A["Wg_lo"] + dc * E:_W0A["Wg_lo"] + (dc + 1) * E] = \
            Wg_lo[dc * P:(dc + 1) * P, :]
    w0a[:, _W0A["ones"]:_W0A["ones"] + P] = np.ones((P, P), BFNP)
    put("bq_row", 0, bq_f)
    put("bo", slice(0, E), bo_p)

    w0b = np.zeros((P, _W0B_COLS), BFNP)
    for dc in range(DC):
        w0b[:, dc * D:(dc + 1) * D] = \
            np.asarray(Wq_f, np.float32)[dc * P:(dc + 1) * P, :].astype(BFNP)
        w0b[:, DC * D + dc * D:DC * D + (dc + 1) * D] = \
            np.asarray(Wo_f, np.float32)[dc * P:(dc + 1) * P, :].astype(BFNP)

    def pack_chunks(x, n):  # [512, N] -> [128, 4*N] (dc-major columns)
        out = np.empty((P, DC * n), x.dtype)
        for dc in range(DC):
            out[:, dc * n:(dc + 1) * n] = x[dc * P:(dc + 1) * P, :]
        return np.ascontiguousarray(out)

    def pack_blocks(x, nblk, blkcols):  # [512, N] -> [128, nblk*(4*blkcols)]
        # block j = [dc0 cols | dc1 cols | dc2 cols | dc3 cols]
        out = np.empty((P, DC * nblk * blkcols), x.dtype)
        for j in range(nblk):
            for dc in range(DC):
                off = j * DC * blkcols + dc * blkcols
                out[:, off:off + blkcols] = \
                    x[dc * P:(dc + 1) * P, j * blkcols:(j + 1) * blkcols]
        return np.ascontiguousarray(out)

    shared = {"w0a": np.ascontiguousarray(w0a), "w0b": np.ascontiguousarray(w0b)}
    in_maps = []
    for b in range(4):
        kT = np.asarray(key[b], np.float32).T.astype(BFNP)     # [512, 2048]
        vT = np.asarray(value[b], np.float32).T.astype(BFNP)
        kTn = pack_blocks(kT, 4, HD)      # ncH-major blocks of 512 keys
        vTn = pack_blocks(vT, NKC, P)     # kc-major blocks of 128 keys
        for h in range(2):
            qhi, qlo = hilo(np.asarray(query[b][h * NQ:(h + 1) * NQ, :]).T)
            in_maps.append({"kTn": kTn, "vTn": vTn,
                            "qhi": pack_chunks(qhi, NQ),
                            "qlo": pack_chunks(qlo, NQ), **shared})
    return in_maps


def kernel(query, key, value, Wg, Wk, bk, Wv, bv, Wq, bq, Wo, bo):
    in_maps = make_in_maps(query, key, value, Wg, Wk, bk, Wv, bv, Wq, bq, Wo, bo)
    nc = get_program()
    res = bass_utils.run_bass_kernel_spmd(nc, in_maps, core_ids=list(range(8)))
    outs = [np.asarray(res.results[c]["out"], np.float32) for c in range(8)]
    return np.concatenate(outs, axis=0).reshape(4, T, D)
